# revision 36
# baseline (speedup 1.0000x reference)
"""Expert-parallel MoE "behind" block + residual on 8 Trainium2 NeuronCores.

Reference computation (fp32):
    front      = inputs[:E*C].reshape(E, C, D_IN)
    expert_out = einsum("ecd,edm->ecm", front, expert_w) + expert_b
    combined   = einsum("sec,ecm->sm", combine_weights, expert_out)
    resid      = inputs[E*C:] @ residual_w + residual_b
    out        = combined * w0[:, None] + resid * w1[:, None]

Sharding (8 cores):
  Stage 1 (expert-parallel, bf16): core e computes eo_e = front_e @ W_e
  [C, D_OUT] in two c-halves; each half is copied out of PSUM twice — once
  to bf16 (kept locally) and once to fp8 e4m3 with a 1/4 scale — and the
  fp8 copy is AllGathered (halved collective payload vs bf16: ~38 us/chunk).
  Stage 3 (token-parallel residual): fp8 e4m3 DoubleRow as before.
  Stage 2 (token-parallel combine): ALL 16 k-blocks run fp8 e4m3 DoubleRow
  (2 k-rows/cycle) against the AllGathered fp8 eo.  Two accuracy devices
  make this fit the 2e-2 rel-l2 bar (measured 1.664e-2 baseline budget):

  * Mean-centering: cw is uniform[0,1); the device matmuls use
    v = (cw - 0.5)*w0 (rms halved), which halves BOTH fp8 error terms
    (q(v)·eo and v·q(eo)); the removed mean is an exact host-side rank-1
    term  0.5*w0[s] * colsum(eo)[m]  with colsum(eo) = sum_e
    (colsum front_e) @ W_e computed on host in float64 (67 MFLOP, same
    spirit as the existing exact expert_b/residual_b foldbacks).
  * Local-bf16 twins: each core's OWN two eo chunks (half h, rank r) never
    need the collective — they are read from the local bf16 eo_half and
    contracted in bf16.  SPMD runs one program on all cores, so the fp8
    slots for those two blocks still execute but with their cw data zeroed
    host-side (exact zero contribution), and two extra bf16 blocks carry
    the real values.  Costs 2x4.2 us of zero-matmuls; removes the two
    largest error terms per core.

  Default config runs ALL 16 blocks fp8 (LOCAL_BF16=0): measured rel-l2
  1.8876e-2 on HW, bit-identical to the numpy emulation and deterministic
  across runs (fixed seed-0 inputs; the metric averages 4M outputs, so it
  is also insensitive to input resampling).  LOCAL_BF16=2 gives 1.767e-2
  at +25 us.

All DRAM operands are host-swizzled partition-major as before (fat DMA
descriptors).  Perf notes (measured on these trn2 cores):
  * Every 512-col MM cadences at exactly 263 ns (512 cycles at the
    sustained 13/16-throttled 1.95 GHz clock), bf16 and fp8-DR alike; the
    kernel is cadence-bound: 894 MMs = 235 us + ~12 us boot head + ~2 us
    gaps + ~2 us tail = 252-262 us (vs 316-326 us baseline).  Some runs
    draw the full 2.4 GHz clock (no SW throttle) — the ring balancing
    below is what lets those run supply-clean.
  * Stage-1 ft/we tile loads alternate between the two HWDGE rings per
    block: the we stream alone is ~8.6 us/MB-tile x 16 = 137 us, which
    out-runs stage-1's PE time on one ring.  S1SUB=4 (4 k-subtiles per
    DMA) halves descriptor counts; ri/rw/cw ship as 2-block granules.
  * eoag loads alternate sync/scalar, with each queue's FIRST eoag pinned
    after stage-1's last PSUM copies — an AG-gated dispatch hoisted ahead
    of them head-of-line blocks the engine FIFO, stalls PSUM turnaround,
    and serializes the collective across all cores (measured 68 us).
  * The two fp8 AllGathers run serially on the CC engine (~100 us
    combined); stage-3 splits 6+2 around stage-2 half 0 to cover their
    tails.
  * The last 6 combine blocks run bank-major so each PSUM bank's output
    copy + DMA hides behind the remaining matmuls.
  * 16 warmup matmuls release the PE HAM clock gate during the ~12 us
    boot+DMA head (N_WARM=0 measured worse: mid-stage-1 HAM wobble).
  * _pairskip drops redundant Ldweights.

Env knobs: TRN_LOCAL_BF16=0 (default; 2 = bf16 twins, lower error),
TRN_FP8_S3=1, TRN_WARM=16, TRN_SKIP_LDW=1, TRN_PAIR_SKIP=1.
"""

import json
import os
import numpy as np
import ml_dtypes

E, C, D_IN, D_OUT = 8, 1024, 4096, 1024
B, S = 2, 2048
TOK = B * S                 # 4096 tokens
N_CORES = 8
S_LOC = TOK // N_CORES      # 512 tokens per core
CH = C // 2                 # c-half = 512
BF16 = ml_dtypes.bfloat16
F8 = ml_dtypes.float8_e4m3

LDW_OPT = os.environ.get("TRN_LDW_OPT", "0") == "1"
SKIP_LDW = os.environ.get("TRN_SKIP_LDW", "1") == "1"
PAIR_SKIP = os.environ.get("TRN_PAIR_SKIP", "1") == "1"
FP8_S3 = os.environ.get("TRN_FP8_S3", "1") == "1"
N_WARM = int(os.environ.get("TRN_WARM", "28"))
SPLIT_COPY = os.environ.get("TRN_SPLIT_COPY", "1") == "1"
# 2 = each core contracts its own two eo chunks in bf16 (fp8 slots zeroed);
# 0 = all 16 combine blocks in fp8 (rel-l2 1.888e-2 vs 1.767e-2, both
# deterministic on the fixed seed-0 inputs; 0 saves ~25 us of PE time)
LOCAL_BF16 = int(os.environ.get("TRN_LOCAL_BF16", "0"))
# fp8 e4m3 operand scalings (products exact):
FP8_S = 4.0                 # stage 3: riT*(1/S), rw*S
S2_SCALE = 4.0              # stage 2: cw8*S, eo8*(1/S)

_prog_cache = {}


def _pairskip_ldweights_json(d):
    """Delete Ldweights that reload the weights already in the PE array.

    bass unconditionally emits Ldweights+Matmult for every matmul, so the
    mm_pair weight-reuse never took effect.  Measured on HW: a 512-row bf16
    MM whose weights must load first has a 263 ns cadence vs 216 ns when the
    array already holds them.  Emulate the array state over the PE queue and
    drop any Ldweights whose weights AP exactly matches the one currently
    loaded; its semaphore wait (rare) moves to the next PE instruction or an
    EventSemaphore.
    """
    n_del = 0
    for fn in d["functions"]:
        for blk in fn["blocks"]:
            out, loaded, carry_wait = [], None, []
            for ins in blk["instructions"]:
                op = ins["opcode"]
                if op == "Ldweights":
                    sig = json.dumps(ins["ins"][0], sort_keys=True)
                    if sig == loaded:
                        w = (ins.get("sync_info") or {}).get("on_wait") or []
                        carry_wait.extend(w)
                        n_del += 1
                        continue
                    loaded = sig
                elif op == "Matmult" and carry_wait:
                    si = ins.get("sync_info") or {"on_update": [], "on_wait": []}
                    ins["sync_info"] = si
                    if not (si.get("on_wait") or []):
                        si["on_wait"] = [carry_wait.pop(0)]
                    for w in carry_wait:
                        out.append({
                            "debug": ins.get("debug", 0), "engine": "PE",
                            "ins": [], "outs": [],
                            "name": ins["name"] + f"_cw{len(out)}",
                            "opcode": "EventSemaphore",
                            "sync_info": {"on_update": [], "on_wait": [w]},
                        })
                    carry_wait = []
                out.append(ins)
            assert not carry_wait
            blk["instructions"] = out
    return d, n_del


def _patch_ldw_opt():
    from concourse import bass_utils
    if getattr(bass_utils, "_ldw_opt_patched", False):
        return
    orig = bass_utils.run_command

    def patched(argv, **kw):
        argv = ["--enable-ldw-opt=true" if a == "--enable-ldw-opt=false" else a
                for a in argv]
        return orig(argv, **kw)

    bass_utils.run_command = patched
    bass_utils._ldw_opt_patched = True


def _build(ldw_opt):
    import concourse.bass as bass  # noqa: F401
    import concourse.mybir as mybir
    from concourse import bacc
    from concourse.tile import TileContext, add_dep_helper

    dt = mybir.dt
    io_dt = dt.bfloat16

    nc = bacc.Bacc("TRN2", target_bir_lowering=False, debug=False, num_devices=N_CORES)

    s3_dt = dt.float8e4 if FP8_S3 else io_dt
    fT = nc.declare_dram_parameter("fT", [128, D_IN // 128 * C], io_dt, isOutput=False)
    we = nc.declare_dram_parameter("we", [128, D_IN // 128 * D_OUT], io_dt, isOutput=False)
    cwT8 = nc.declare_dram_parameter("cwT8", [128, E * C // 128 * S_LOC], dt.float8e4,
                                     isOutput=False)
    cwT16 = (nc.declare_dram_parameter("cwT16", [128, LOCAL_BF16 * 4 * S_LOC], io_dt,
                                       isOutput=False) if LOCAL_BF16 else None)
    riT = nc.declare_dram_parameter("riT", [128, D_IN // 128 * S_LOC], s3_dt, isOutput=False)
    rw = nc.declare_dram_parameter("rw", [128, D_IN // 128 * D_OUT], s3_dt, isOutput=False)
    out = nc.declare_dram_parameter("out", [S_LOC, D_OUT], dt.float32, isOutput=True)

    # variant tag so differently-compiled builds never share a jax cache entry
    nc.dram_tensor(
        f"variant_v2_{int(ldw_opt)}_{int(SKIP_LDW)}_{int(PAIR_SKIP)}"
        f"_{int(FP8_S3)}_{N_WARM}_{int(SPLIT_COPY)}_loc{LOCAL_BF16}",
        [1, 1], dt.float32)

    ag_in = [nc.dram_tensor(f"ag_in{h}", [128, 4 * D_OUT], dt.float8e4) for h in range(2)]
    ag_out = [nc.dram_tensor(f"ag_out{h}", [N_CORES * 128, 4 * D_OUT], dt.float8e4,
                             addr_space="Shared") for h in range(2)]

    KT = D_IN // 128            # 32 contraction tiles
    SUB = 4                     # k-subtiles per DMA'd block
    NBLK = KT // SUB            # 8 stage-3 blocks
    ECT = (E * C) // 128        # 64 combine contraction tiles
    NFREE = 512                 # PSUM bank cap: 512 fp32 out elements = 2 KiB
    NJ = D_OUT // NFREE

    S1SUB = 4                   # k-subtiles per stage-1 DMA'd block: 4KB/8KB
    S1BLK = KT // S1SUB         # descriptors halve ring time vs 2-subtile
    B_FT, B_WE, B_RI, B_RW, B_CW, B_EOAG = 7, 6, 4, 4, 3, 8
    NPRE = 1                    # stage-3/2 block-pairs prefetched during stage 1
    with TileContext(nc) as tc:
        with tc.tile_pool(name="p_ft", bufs=B_FT) as p_ft, \
             tc.tile_pool(name="p_we", bufs=B_WE) as p_we, \
             tc.tile_pool(name="p_ri", bufs=B_RI) as p_ri, \
             tc.tile_pool(name="p_rw", bufs=B_RW) as p_rw, \
             tc.tile_pool(name="p_cw", bufs=B_CW) as p_cw, \
             tc.tile_pool(name="p_cwl", bufs=2) as p_cwl, \
             tc.tile_pool(name="p_eoag", bufs=B_EOAG) as p_eoag, \
             tc.tile_pool(name="p_eo", bufs=2) as p_eo, \
             tc.tile_pool(name="p_eo8", bufs=2) as p_eo8, \
             tc.tile_pool(name="p_out", bufs=1) as p_out, \
             tc.tile_pool(name="p_warm", bufs=1) as p_warm, \
             tc.tile_pool(name="psum", bufs=1, space="PSUM") as p_ps:

            def mm_pair(psrow, lhsT_ap, rhs_of_j, start, stop, perf_mode=None):
                """Two matmuls sharing one stationary operand: the second
                skips its LDWEIGHTS and is order-pinned after the first."""
                prev = None
                for j in range(NJ):
                    m = nc.tensor.matmul(psrow[j], lhsT_ap, rhs_of_j(j),
                                         start=start, stop=stop,
                                         perf_mode=perf_mode)
                    if j > 0 and SKIP_LDW:
                        m.ins.ldweights = False
                        add_dep_helper(m.ins, prev.ins, False, "weight-reuse pair order")
                    prev = m

            def psum_tiles(tagp):
                return [[p_ps.tile([128, NFREE], dt.float32,
                                   name=f"{tagp}_{i}_{j}", tag=f"ps_{i}_{j}")
                         for j in range(NJ)] for i in range(4)]

            # ------------- Warmup: release the PE HAM clock gate -------------
            if N_WARM:
                warm_t = p_warm.tile([128, 128 + NFREE], io_dt)
                nc.vector.memset(warm_t, 0)
                warm_ps = p_ps.tile([128, NFREE], dt.float32,
                                    name="warm_ps", tag="ps_0_0")
                for _ in range(N_WARM):
                    nc.tensor.matmul(warm_ps, warm_t[:, :128],
                                     warm_t[:, 128:128 + NFREE],
                                     start=True, stop=True)

            ri_tiles, rw_tiles, cw_tiles = {}, {}, {}
            unflat = lambda t, n: t.rearrange("p (n d) -> p n d", n=n)

            # 2-block granule loads: each partition's data for consecutive
            # blocks is contiguous in the swizzled DRAM layouts, so pairing
            # blocks halves the descriptor count on the rings
            def load_ri(blk):
                b0 = blk - blk % 2
                t = p_ri.tile([128, 2 * SUB * S_LOC], s3_dt, tag="ri", name=f"ri_{b0}")
                nc.sync.dma_start(
                    out=t, in_=riT[:, b0 * SUB * S_LOC:(b0 + 2) * SUB * S_LOC])
                v = unflat(t, 2 * SUB)
                ri_tiles[b0] = v[:, :SUB]
                ri_tiles[b0 + 1] = v[:, SUB:]

            def load_rw(blk, q=None):
                b0 = blk - blk % 2
                t = p_rw.tile([128, 2 * SUB * D_OUT], s3_dt, tag="rw", name=f"rw_{b0}")
                d = (q or nc.scalar).dma_start(
                    out=t, in_=rw[:, b0 * SUB * D_OUT:(b0 + 2) * SUB * D_OUT])
                v = unflat(t, 2 * SUB)
                rw_tiles[b0] = v[:, :SUB]
                rw_tiles[b0 + 1] = v[:, SUB:]
                return d

            def load_cw(blk):
                b0 = blk - blk % 2
                t = p_cw.tile([128, 2 * SUB * S_LOC], dt.float8e4, tag="cw",
                              name=f"cw8_{b0}")
                nc.sync.dma_start(
                    out=t, in_=cwT8[:, b0 * SUB * S_LOC:(b0 + 2) * SUB * S_LOC])
                v = unflat(t, 2 * SUB)
                cw_tiles[b0] = v[:, :SUB]
                cw_tiles[b0 + 1] = v[:, SUB:]

            # ------------- Stage 1: eo_e = fT.T @ we, by c-halves ------------
            last_we = [None]
            last_copy = {"dve": None, "act": None}
            eo16 = {}
            s1_tiles = {}

            def load_s1(ch, blk):
                if (ch, blk) in s1_tiles:
                    return s1_tiles[(ch, blk)]
                f0 = (ch * S1BLK + blk) * (S1SUB * CH)
                w0_ = blk * (S1SUB * D_OUT)
                ft_f = p_ft.tile([128, S1SUB * CH], io_dt, tag="ft", name=f"ft_{ch}_{blk}")
                we_f = p_we.tile([128, S1SUB * D_OUT], io_dt, tag="we", name=f"we_{ch}_{blk}")
                if ch == 0 and blk == 0:
                    # the first block gates the whole pipeline: balance its
                    # 1.5 MB across the two fast HWDGE rings (~6.5 us each
                    # at ~116 GB/s).  Keep it OFF the gpsimd SWDGE ring
                    # (~25 GB/s — routing a 256 KB ft half there measured
                    # first-matmul at ~23 us instead of ~13).
                    nc.sync.dma_start(out=ft_f, in_=fT[:, f0:f0 + S1SUB * CH])
                    last_we[0] = nc.scalar.dma_start(
                        out=we_f[0:96, :],
                        in_=we[0:96, w0_:w0_ + S1SUB * D_OUT])
                    nc.sync.dma_start(
                        out=we_f[96:128, :],
                        in_=we[96:128, w0_:w0_ + S1SUB * D_OUT])
                else:
                    # balance the two HWDGE rings: the we stream alone is
                    # ~8.6us/tile (1 MB at ~116 GB/s) x 16 = 137us, which
                    # binds stage 1 on one ring; alternating ft/we between
                    # the rings puts ~103us on each
                    qa, qb = ((nc.sync, nc.scalar) if blk % 2 == ch
                              else (nc.scalar, nc.sync))
                    qa.dma_start(out=ft_f, in_=fT[:, f0:f0 + S1SUB * CH])
                    last_we[0] = qb.dma_start(
                        out=we_f, in_=we[:, w0_:w0_ + S1SUB * D_OUT])
                s1_tiles[(ch, blk)] = (unflat(ft_f, S1SUB), unflat(we_f, S1SUB))
                return s1_tiles[(ch, blk)]

            for ch in range(2):
                psums = psum_tiles(f"s1h{ch}")
                for blk in range(S1BLK):
                    # pre-issue the FIRST stage-3/2 operand pairs late in
                    # half 0, one per block (early half-0 slots would steal
                    # ring time from the ft/we ramp; half-1 slots dilute the
                    # boundary supply; later pairs arrive in time via the
                    # stage-2/3 emission path)
                    if ch == 0 and S1BLK - 1 - 3 * NPRE <= blk < S1BLK - 1:
                        k, r = divmod(S1BLK - 1 - blk - 1, 3)
                        if k < NPRE:
                            # 2-block granules: k-th pre-issue covers blocks
                            # 2k, 2k+1 of each operand
                            (load_ri,
                             lambda b: load_rw(b, q=nc.scalar),
                             load_cw)[r](2 * k)
                    # hoist half-1's first blocks into late half 0 so their
                    # ring positions precede the boundary (covers a 2-4us
                    # PE stall while ch1-blk0/1 would otherwise still load)
                    if ch == 0 and blk in (3, 5):
                        load_s1(1, 0 if blk == 3 else 1)
                    ft_t, we_t = load_s1(ch, blk)
                    if blk < S1BLK - 1:
                        for sub in range(S1SUB):
                            kt = blk * S1SUB + sub
                            for i in range(4):
                                mm_pair(psums[i],
                                        ft_t[:, sub, i * 128:(i + 1) * 128],
                                        lambda j, sub=sub: we_t[:, sub, j * NFREE:(j + 1) * NFREE],
                                        start=(kt == 0), stop=False)
                eo8_half = p_eo8.tile([128, 4 * D_OUT], dt.float8e4, tag="eo8",
                                      name=f"eo8_{ch}")
                if LOCAL_BF16:
                    eo_half = p_eo.tile([128, 4 * D_OUT], io_dt, tag="eo",
                                        name=f"eo_{ch}")
                    eo16[ch] = unflat(eo_half, 4)
                # last block: finish each PSUM bank in turn; copy to bf16
                # (DVE, kept local) and scaled fp8 (ACT, AllGathered)
                for i in range(4):
                    for j in range(NJ):
                        for sub in range(S1SUB):
                            nc.tensor.matmul(
                                psums[i][j],
                                ft_t[:, sub, i * 128:(i + 1) * 128],
                                we_t[:, sub, j * NFREE:(j + 1) * NFREE],
                                start=False, stop=(sub == S1SUB - 1))
                        o0 = i * D_OUT + j * NFREE
                        if LOCAL_BF16:
                            nc.vector.tensor_copy(
                                out=eo_half[:, o0:o0 + NFREE], in_=psums[i][j])
                        # split the fp8 scaled copies across DVE and ACT so the
                        # 8-bank turnaround doesn't serialize on one engine
                        # (single-engine measured a 4.4us PE stall + HAM
                        # re-throttle at the half boundary)
                        if SPLIT_COPY and j % 2 == 0 and not LOCAL_BF16:
                            last_copy["dve"] = nc.vector.tensor_scalar_mul(
                                eo8_half[:, o0:o0 + NFREE], psums[i][j],
                                1.0 / S2_SCALE)
                        else:
                            last_copy["act"] = nc.scalar.mul(
                                eo8_half[:, o0:o0 + NFREE], psums[i][j],
                                1.0 / S2_SCALE)
                    # SWDGE queue keeps these late-gated writes out of the
                    # HWDGE FIFOs (head-of-line blocking of operand loads)
                    if i % 2:
                        nc.gpsimd.dma_start(
                            out=ag_in[ch][:, (i - 1) * D_OUT:(i + 1) * D_OUT],
                            in_=eo8_half[:, (i - 1) * D_OUT:(i + 1) * D_OUT])
                # chunked fp8 AllGather (half the bytes of the bf16 one)
                nc.gpsimd.collective_compute(
                    "AllGather", mybir.AluOpType.bypass,
                    replica_groups=[list(range(N_CORES))],
                    ins=[ag_in[ch][:].opt()], outs=[ag_out[ch][:].opt()])

            # ------------- Stages 2+3, interleaved ---------------------------
            # Emission order: s3 blocks 0-5 (covers AG0 tail), local-bf16 twin
            # of half 0, s2 fp8 blocks 0-7, s3 blocks 6-7 (covers AG1 tail),
            # local-bf16 twin of half 1, s2 fp8 blocks 8-15 (block 15
            # finishes: PSUM->SBUF copies + output DMA).
            psums = psum_tiles("s23")
            s3_dr = s3_dt == dt.float8e4
            out_sb = p_out.tile([128, 4, D_OUT], dt.float32)
            eoag_state = {"prev": None}
            S3_SPLIT = 6

            def s2_local_block(h):
                # this core's own eo chunk, bf16, no collective dependency
                t = p_cwl.tile([128, SUB * S_LOC], io_dt, tag="cwl", name=f"cw16_{h}")
                nc.sync.dma_start(
                    out=t, in_=cwT16[:, h * SUB * S_LOC:(h + 1) * SUB * S_LOC])
                cw_t = unflat(t, SUB)
                for sub in range(SUB):
                    for i in range(4):
                        mm_pair(psums[i],
                                cw_t[:, sub, i * 128:(i + 1) * 128],
                                lambda j, sub=sub: eo16[h][:, sub, j * NFREE:(j + 1) * NFREE],
                                start=False, stop=False)

            def load_eoag(blk):
                half = blk // 8              # ag chunk this block reads
                rk = blk % 8                 # rank whose eo chunk this is
                if blk not in cw_tiles:
                    load_cw(blk)
                eo_f = p_eoag.tile([128, SUB * D_OUT], dt.float8e4, tag="eoag",
                                   name=f"eoag_{blk}")
                # alternate rings: stage-1's ft/we streams are done by now, so
                # the sync ring is free to carry half the eoag supply
                q = nc.scalar if blk % 2 else nc.sync
                eoag_dma = q.dma_start(
                    out=eo_f, in_=ag_out[half][rk * 128:(rk + 1) * 128, :])
                # pin each queue's first eoag after stage-1's LAST PSUM->SBUF
                # copies and the operand loads, then keep block order within
                # the queue.  Without this the scheduler hoists an AG-gated
                # dispatch ahead of the half-1 copies: the engine FIFO
                # head-of-line blocks on the collective, PSUM turnaround
                # stalls, and every core's stage-1 tail (so the collective
                # itself) serializes (measured 68us PE stall).
                key = "prev_s" if blk % 2 else "prev_y"
                prev = eoag_state.get(key)
                if prev is not None:
                    add_dep_helper(eoag_dma.ins, prev.ins, False,
                                   "eoag in block order per queue")
                else:
                    for root in (last_copy["dve"], last_copy["act"],
                                 eoag_state.get("root") or last_we[0]):
                        if root is not None:
                            add_dep_helper(eoag_dma.ins, root.ins, False,
                                           "first eoag after stage-1 tail")
                eoag_state[key] = eoag_dma
                return unflat(eo_f, SUB)

            def s2_block(blk):
                eo_t = load_eoag(blk)
                cw_t = cw_tiles[blk]
                for s2i in range(SUB // 2):
                    for i in range(4):
                        mm_pair(psums[i],
                                cw_t[:, 2 * s2i:2 * s2i + 2, i * 128:(i + 1) * 128],
                                lambda j, s2i=s2i: eo_t[:, 2 * s2i:2 * s2i + 2,
                                                        j * NFREE:(j + 1) * NFREE],
                                start=False, stop=False,
                                perf_mode=mybir.MatmulPerfMode.DoubleRow)
                return cw_t, eo_t

            def s2_tail(blks):
                # bank-major sweep over the last blocks: each (i, j0/j1) bank
                # pair finishes ~4.2 us apart, so its PSUM->SBUF copies (split
                # DVE/ACT) and output DMAs (split sync/scalar) hide behind the
                # remaining matmuls instead of stacking up after the last one
                tiles = []
                for b in blks:
                    eo_t = load_eoag(b)
                    tiles.append((cw_tiles[b], eo_t))
                for i in range(4):
                    for bi, (cw_t, eo_t) in enumerate(tiles):
                        for s2i in range(SUB // 2):
                            mm_pair(psums[i],
                                    cw_t[:, 2 * s2i:2 * s2i + 2, i * 128:(i + 1) * 128],
                                    lambda j, s2i=s2i, eo_t=eo_t: eo_t[:, 2 * s2i:2 * s2i + 2,
                                                                      j * NFREE:(j + 1) * NFREE],
                                    start=False,
                                    stop=(bi == len(tiles) - 1 and s2i == SUB // 2 - 1),
                                    perf_mode=mybir.MatmulPerfMode.DoubleRow)
                    for j in range(NJ):
                        if SPLIT_COPY and j % 2:
                            nc.scalar.copy(out_sb[:, i, j * NFREE:(j + 1) * NFREE],
                                           psums[i][j])
                        else:
                            nc.vector.tensor_copy(
                                out=out_sb[:, i, j * NFREE:(j + 1) * NFREE],
                                in_=psums[i][j])
                        q = nc.scalar if (SPLIT_COPY and j % 2) else nc.sync
                        q.dma_start(
                            out=out[i * 128:(i + 1) * 128, j * NFREE:(j + 1) * NFREE]
                                .rearrange("(n p) d -> p n d", p=128),
                            in_=out_sb[:, i:i + 1, j * NFREE:(j + 1) * NFREE])

            def s3_blocks(b0, b1, start):
                if b0 == 0:
                    # issue the remaining stage-3 operand loads now, BEFORE any
                    # AG-gated eoag dispatch enters the rings (ring descriptors
                    # process in order; anything queued behind a gated eoag
                    # waits for the collective)
                    for blk in range(NBLK):
                        if blk not in ri_tiles:
                            load_ri(blk)
                        if blk not in rw_tiles:
                            eoag_state["root"] = load_rw(blk)
                for blk in range(b0, b1):
                    if blk not in ri_tiles:
                        load_ri(blk)
                    ri_t = ri_tiles[blk]
                    if blk not in rw_tiles:
                        eoag_state["root"] = load_rw(blk)
                    rw_t = rw_tiles[blk]
                    if s3_dr:
                        for s2 in range(SUB // 2):
                            for i in range(4):
                                mm_pair(psums[i],
                                        ri_t[:, 2 * s2:2 * s2 + 2, i * 128:(i + 1) * 128],
                                        lambda j, s2=s2: rw_t[:, 2 * s2:2 * s2 + 2,
                                                              j * NFREE:(j + 1) * NFREE],
                                        start=(start and blk == b0 and s2 == 0),
                                        stop=False,
                                        perf_mode=mybir.MatmulPerfMode.DoubleRow)
                    else:
                        for sub in range(SUB):
                            for i in range(4):
                                mm_pair(psums[i],
                                        ri_t[:, sub, i * 128:(i + 1) * 128],
                                        lambda j, sub=sub: rw_t[:, sub, j * NFREE:(j + 1) * NFREE],
                                        start=(start and blk == b0 and sub == 0),
                                        stop=False)

            s3_blocks(0, S3_SPLIT, start=True)       # covers AG0's tail
            if LOCAL_BF16:
                s2_local_block(0)                    # AG-independent filler
            for blk in range(8):                     # stage-2 fp8 half 0
                s2_block(blk)
            s3_blocks(S3_SPLIT, NBLK, start=False)   # covers AG1's tail
            if LOCAL_BF16 == 2:
                s2_local_block(1)                    # AG-independent filler
            for blk in range(8, 10):                 # stage-2 fp8 half 1
                s2_block(blk)
            s2_tail(list(range(10, ECT // SUB)))     # bank-major finish

    nc.finalize()
    if PAIR_SKIP:
        d, n_del = _pairskip_ldweights_json(json.loads(nc.to_json_bytes()))
        fused = json.dumps(d).encode()
        nc.to_json_bytes = lambda: fused
    return nc


def _get_prog(ldw_opt):
    key = (ldw_opt,)
    if key not in _prog_cache:
        if ldw_opt:
            _patch_ldw_opt()
        _prog_cache[key] = _build(ldw_opt)
    return _prog_cache[key]


def _swz(a, nblk, nsub):
    """[nblk*nsub*128, d] contraction-major -> partition-major [128, nblk*nsub*d]
    with each partition's data contiguous (fat DMA descriptors)."""
    d = a.shape[1]
    return np.ascontiguousarray(
        a.reshape(nblk, nsub, 128, d).transpose(2, 0, 1, 3).reshape(128, nblk * nsub * d))


def _prep_in_maps(inputs, expert_w, residual_w, combine_weights, residual_weight):
    np_dt = BF16
    front = inputs[:E * C].reshape(E, C, D_IN)
    resid = inputs[E * C:]                       # [TOK, D_IN]
    rwt = residual_weight.reshape(TOK, 2)
    w0, w1 = rwt[:, 0], rwt[:, 1]

    s3_np = F8 if FP8_S3 else np_dt
    rw_scale = FP8_S if FP8_S3 else 1.0
    rw_sw = _swz((residual_w * rw_scale).astype(s3_np), 8, 4)            # [128, 8*4*1024]
    resid_s = resid * (w1[:, None] / rw_scale)   # fold w1 and 1/S (fp32)
    in_maps = []
    for r in range(N_CORES):
        sl = slice(r * S_LOC, (r + 1) * S_LOC)
        fT = front[r].T.astype(np_dt)                                    # [D_IN, C]
        # fT layout [128, (ch, blk, sub, c)]: the kernel reads c-halves
        fT_sw = np.ascontiguousarray(
            fT.reshape(16, 2, 128, 2, CH).transpose(2, 3, 0, 1, 4).reshape(128, -1))
        we_sw = _swz(expert_w[r].astype(np_dt), 16, 2)                   # [128, 16*2*1024]
        # centered combine weights: v = (cw - 0.5) * w0; the removed mean is
        # added back exactly on the host (rank-1 term, see _run)
        cw_c = (combine_weights[sl] - 0.5) * w0[sl, None, None]          # [S_LOC, E, C]
        # contraction rows ordered (c-half chunk, expert, c-within-half) to
        # match the chunked AllGather's concatenation
        cwT_f32 = (cw_c.reshape(S_LOC, E, 2, CH).transpose(2, 1, 3, 0)
                   .reshape(E * C, S_LOC))
        cw8_src = cwT_f32 * S2_SCALE
        if LOCAL_BF16:
            # this core's own chunks run through the local bf16 path; zero
            # their fp8 slots (the SPMD program still executes them)
            loc_blocks = [r, 8 + r][:LOCAL_BF16]
            cw8_src = cw8_src.copy()
            for b in loc_blocks:
                cw8_src[b * 512:(b + 1) * 512] = 0.0
            cw16 = np.concatenate(
                [cwT_f32[b * 512:(b + 1) * 512]
                 for b in sorted(loc_blocks)], axis=0)
            cwT16_sw = _swz(cw16.astype(np_dt), LOCAL_BF16, 4)
        cwT8_sw = _swz(cw8_src.astype(F8), 16, 4)                        # [128, 16*4*512]
        riT_sw = _swz(resid_s[sl].T.astype(s3_np), 8, 4)                 # [128, 8*4*512]
        m = {"fT": fT_sw, "we": we_sw, "cwT8": cwT8_sw, "riT": riT_sw, "rw": rw_sw}
        if LOCAL_BF16:
            m["cwT16"] = cwT16_sw
        in_maps.append(m)
    return in_maps


def _run(inputs, expert_w, expert_b, residual_w, residual_b,
         combine_weights, residual_weight, ldw_opt=None, trace=False, mode=None):
    import jax
    try:
        if jax.config.jax_compilation_cache_dir is None:
            jax.config.update("jax_compilation_cache_dir", "/tmp/jax_cache_trn_moe")
            jax.config.update("jax_persistent_cache_min_compile_time_secs", 0.5)
    except Exception:
        pass
    from concourse.bass_utils import run_bass_kernel_spmd

    ldw_opt = LDW_OPT if ldw_opt is None else ldw_opt
    inputs = np.asarray(inputs, dtype=np.float32)
    expert_w = np.asarray(expert_w, dtype=np.float32)
    expert_b = np.asarray(expert_b, dtype=np.float32)
    residual_w = np.asarray(residual_w, dtype=np.float32)
    residual_b = np.asarray(residual_b, dtype=np.float32)
    combine_weights = np.asarray(combine_weights, dtype=np.float32)
    residual_weight = np.asarray(residual_weight, dtype=np.float32)

    nc = _get_prog(ldw_opt)
    in_maps = _prep_in_maps(inputs, expert_w, residual_w, combine_weights,
                            residual_weight)
    res = run_bass_kernel_spmd(nc, in_maps, list(range(N_CORES)), trace=trace)
    out = np.concatenate([res.results[r]["out"] for r in range(N_CORES)], axis=0)

    rwt = residual_weight.reshape(TOK, 2)
    # exact centering term: 0.5 * w0[s] * colsum(front @ W)[m], float64
    front = inputs[:E * C].reshape(E, C, D_IN)
    G = np.zeros(D_OUT, np.float64)
    for e in range(E):
        G += front[e].sum(axis=0).astype(np.float64) @ expert_w[e].astype(np.float64)
    out = out + (0.5 * rwt[:, 0:1]) * G[None, :].astype(np.float32)
    # exact bias contributions (zero in practice, but keep the math honest)
    if residual_b.any():
        out = out + rwt[:, 1:2] * residual_b[None, :]
    if expert_b.any():
        cs = combine_weights.sum(axis=2)                    # [TOK, E]
        out = out + rwt[:, 0:1] * (cs @ expert_b)
    return out.reshape(B, S, D_OUT).astype(np.float32), res


def kernel(**kw):
    out, _ = _run(**kw)
    return out


# revision 37
# speedup vs baseline: 1.0020x; 1.0020x over previous
"""Expert-parallel MoE "behind" block + residual on 8 Trainium2 NeuronCores.

Reference computation (fp32):
    front      = inputs[:E*C].reshape(E, C, D_IN)
    expert_out = einsum("ecd,edm->ecm", front, expert_w) + expert_b
    combined   = einsum("sec,ecm->sm", combine_weights, expert_out)
    resid      = inputs[E*C:] @ residual_w + residual_b
    out        = combined * w0[:, None] + resid * w1[:, None]

Sharding (8 cores):
  Stage 1 (expert-parallel, bf16): core e computes eo_e = front_e @ W_e
  [C, D_OUT] in two c-halves; each half is copied out of PSUM twice — once
  to bf16 (kept locally) and once to fp8 e4m3 with a 1/4 scale — and the
  fp8 copy is AllGathered (halved collective payload vs bf16: ~38 us/chunk).
  Stage 3 (token-parallel residual): fp8 e4m3 DoubleRow as before.
  Stage 2 (token-parallel combine): ALL 16 k-blocks run fp8 e4m3 DoubleRow
  (2 k-rows/cycle) against the AllGathered fp8 eo.  Two accuracy devices
  make this fit the 2e-2 rel-l2 bar (measured 1.664e-2 baseline budget):

  * Mean-centering: cw is uniform[0,1); the device matmuls use
    v = (cw - 0.5)*w0 (rms halved), which halves BOTH fp8 error terms
    (q(v)·eo and v·q(eo)); the removed mean is an exact host-side rank-1
    term  0.5*w0[s] * colsum(eo)[m]  with colsum(eo) = sum_e
    (colsum front_e) @ W_e computed on host in float64 (67 MFLOP, same
    spirit as the existing exact expert_b/residual_b foldbacks).
  * Local-bf16 twins: each core's OWN two eo chunks (half h, rank r) never
    need the collective — they are read from the local bf16 eo_half and
    contracted in bf16.  SPMD runs one program on all cores, so the fp8
    slots for those two blocks still execute but with their cw data zeroed
    host-side (exact zero contribution), and two extra bf16 blocks carry
    the real values.  Costs 2x4.2 us of zero-matmuls; removes the two
    largest error terms per core.

  Default config runs ALL 16 blocks fp8 (LOCAL_BF16=0): measured rel-l2
  1.8876e-2 on HW, bit-identical to the numpy emulation and deterministic
  across runs (fixed seed-0 inputs; the metric averages 4M outputs, so it
  is also insensitive to input resampling).  LOCAL_BF16=2 gives 1.767e-2
  at +25 us.

All DRAM operands are host-swizzled partition-major as before (fat DMA
descriptors).  Perf notes (measured on these trn2 cores):
  * Every 512-col MM cadences at exactly 263 ns (512 cycles at the
    sustained 13/16-throttled 1.95 GHz clock), bf16 and fp8-DR alike; the
    kernel is cadence-bound: 894 MMs = 235 us + ~12 us boot head + ~2 us
    gaps + ~2 us tail = 252-262 us (vs 316-326 us baseline).  Some runs
    draw the full 2.4 GHz clock (no SW throttle) — the ring balancing
    below is what lets those run supply-clean.
  * Stage-1 ft/we tile loads alternate between the two HWDGE rings per
    block: the we stream alone is ~8.6 us/MB-tile x 16 = 137 us, which
    out-runs stage-1's PE time on one ring.  S1SUB=4 (4 k-subtiles per
    DMA) halves descriptor counts; ri/rw/cw ship as 2-block granules.
  * eoag loads alternate sync/scalar, with each queue's FIRST eoag pinned
    after stage-1's last PSUM copies — an AG-gated dispatch hoisted ahead
    of them head-of-line blocks the engine FIFO, stalls PSUM turnaround,
    and serializes the collective across all cores (measured 68 us).
  * The two fp8 AllGathers run serially on the CC engine (~100 us
    combined); stage-3 splits 6+2 around stage-2 half 0 to cover their
    tails.
  * The last 6 combine blocks run bank-major so each PSUM bank's output
    copy + DMA hides behind the remaining matmuls.
  * 16 warmup matmuls release the PE HAM clock gate during the ~12 us
    boot+DMA head (N_WARM=0 measured worse: mid-stage-1 HAM wobble).
  * _pairskip drops redundant Ldweights.

Env knobs: TRN_LOCAL_BF16=0 (default; 2 = bf16 twins, lower error),
TRN_FP8_S3=1, TRN_WARM=16, TRN_SKIP_LDW=1, TRN_PAIR_SKIP=1.
"""

import json
import os
import numpy as np
import ml_dtypes

E, C, D_IN, D_OUT = 8, 1024, 4096, 1024
B, S = 2, 2048
TOK = B * S                 # 4096 tokens
N_CORES = 8
S_LOC = TOK // N_CORES      # 512 tokens per core
CH = C // 2                 # c-half = 512
BF16 = ml_dtypes.bfloat16
F8 = ml_dtypes.float8_e4m3

LDW_OPT = os.environ.get("TRN_LDW_OPT", "0") == "1"
SKIP_LDW = os.environ.get("TRN_SKIP_LDW", "1") == "1"
PAIR_SKIP = os.environ.get("TRN_PAIR_SKIP", "1") == "1"
FP8_S3 = os.environ.get("TRN_FP8_S3", "1") == "1"
N_WARM = int(os.environ.get("TRN_WARM", "28"))
SPLIT_COPY = os.environ.get("TRN_SPLIT_COPY", "1") == "1"
# 2 = each core contracts its own two eo chunks in bf16 (fp8 slots zeroed);
# 0 = all 16 combine blocks in fp8 (rel-l2 1.888e-2 vs 1.767e-2, both
# deterministic on the fixed seed-0 inputs; 0 saves ~25 us of PE time)
LOCAL_BF16 = int(os.environ.get("TRN_LOCAL_BF16", "0"))
# fp8 e4m3 operand scalings (products exact):
FP8_S = 4.0                 # stage 3: riT*(1/S), rw*S
S2_SCALE = 4.0              # stage 2: cw8*S, eo8*(1/S)

_prog_cache = {}


def _pairskip_ldweights_json(d):
    """Delete Ldweights that reload the weights already in the PE array.

    bass unconditionally emits Ldweights+Matmult for every matmul, so the
    mm_pair weight-reuse never took effect.  Measured on HW: a 512-row bf16
    MM whose weights must load first has a 263 ns cadence vs 216 ns when the
    array already holds them.  Emulate the array state over the PE queue and
    drop any Ldweights whose weights AP exactly matches the one currently
    loaded; its semaphore wait (rare) moves to the next PE instruction or an
    EventSemaphore.
    """
    n_del = 0
    for fn in d["functions"]:
        for blk in fn["blocks"]:
            out, loaded, carry_wait = [], None, []
            for ins in blk["instructions"]:
                op = ins["opcode"]
                if op == "Ldweights":
                    sig = json.dumps(ins["ins"][0], sort_keys=True)
                    if sig == loaded:
                        w = (ins.get("sync_info") or {}).get("on_wait") or []
                        carry_wait.extend(w)
                        n_del += 1
                        continue
                    loaded = sig
                elif op == "Matmult" and carry_wait:
                    si = ins.get("sync_info") or {"on_update": [], "on_wait": []}
                    ins["sync_info"] = si
                    if not (si.get("on_wait") or []):
                        si["on_wait"] = [carry_wait.pop(0)]
                    for w in carry_wait:
                        out.append({
                            "debug": ins.get("debug", 0), "engine": "PE",
                            "ins": [], "outs": [],
                            "name": ins["name"] + f"_cw{len(out)}",
                            "opcode": "EventSemaphore",
                            "sync_info": {"on_update": [], "on_wait": [w]},
                        })
                    carry_wait = []
                out.append(ins)
            assert not carry_wait
            blk["instructions"] = out
    return d, n_del


def _patch_ldw_opt():
    from concourse import bass_utils
    if getattr(bass_utils, "_ldw_opt_patched", False):
        return
    orig = bass_utils.run_command

    def patched(argv, **kw):
        argv = ["--enable-ldw-opt=true" if a == "--enable-ldw-opt=false" else a
                for a in argv]
        return orig(argv, **kw)

    bass_utils.run_command = patched
    bass_utils._ldw_opt_patched = True


def _build(ldw_opt):
    import concourse.bass as bass  # noqa: F401
    import concourse.mybir as mybir
    from concourse import bacc
    from concourse.tile import TileContext, add_dep_helper

    dt = mybir.dt
    io_dt = dt.bfloat16

    nc = bacc.Bacc("TRN2", target_bir_lowering=False, debug=False, num_devices=N_CORES)

    s3_dt = dt.float8e4 if FP8_S3 else io_dt
    fT = nc.declare_dram_parameter("fT", [128, D_IN // 128 * C], io_dt, isOutput=False)
    we = nc.declare_dram_parameter("we", [128, D_IN // 128 * D_OUT], io_dt, isOutput=False)
    cwT8 = nc.declare_dram_parameter("cwT8", [128, E * C // 128 * S_LOC], dt.float8e4,
                                     isOutput=False)
    cwT16 = (nc.declare_dram_parameter("cwT16", [128, LOCAL_BF16 * 4 * S_LOC], io_dt,
                                       isOutput=False) if LOCAL_BF16 else None)
    riT = nc.declare_dram_parameter("riT", [128, D_IN // 128 * S_LOC], s3_dt, isOutput=False)
    rw = nc.declare_dram_parameter("rw", [128, D_IN // 128 * D_OUT], s3_dt, isOutput=False)
    out = nc.declare_dram_parameter("out", [S_LOC, D_OUT], dt.float32, isOutput=True)

    # variant tag so differently-compiled builds never share a jax cache entry
    nc.dram_tensor(
        f"variant_v2_{int(ldw_opt)}_{int(SKIP_LDW)}_{int(PAIR_SKIP)}"
        f"_{int(FP8_S3)}_{N_WARM}_{int(SPLIT_COPY)}_loc{LOCAL_BF16}",
        [1, 1], dt.float32)

    ag_in = [nc.dram_tensor(f"ag_in{h}", [128, 4 * D_OUT], dt.float8e4) for h in range(2)]
    ag_out = [nc.dram_tensor(f"ag_out{h}", [N_CORES * 128, 4 * D_OUT], dt.float8e4,
                             addr_space="Shared") for h in range(2)]

    KT = D_IN // 128            # 32 contraction tiles
    SUB = 4                     # k-subtiles per DMA'd block
    NBLK = KT // SUB            # 8 stage-3 blocks
    ECT = (E * C) // 128        # 64 combine contraction tiles
    NFREE = 512                 # PSUM bank cap: 512 fp32 out elements = 2 KiB
    NJ = D_OUT // NFREE

    S1SUB = 4                   # k-subtiles per stage-1 DMA'd block: 4KB/8KB
    S1BLK = KT // S1SUB         # descriptors halve ring time vs 2-subtile
    B_FT, B_WE, B_RI, B_RW, B_CW, B_EOAG = 7, 6, 4, 4, 3, 8
    NPRE = 1                    # stage-3/2 block-pairs prefetched during stage 1
    with TileContext(nc) as tc:
        with tc.tile_pool(name="p_ft", bufs=B_FT) as p_ft, \
             tc.tile_pool(name="p_we", bufs=B_WE) as p_we, \
             tc.tile_pool(name="p_ri", bufs=B_RI) as p_ri, \
             tc.tile_pool(name="p_rw", bufs=B_RW) as p_rw, \
             tc.tile_pool(name="p_cw", bufs=B_CW) as p_cw, \
             tc.tile_pool(name="p_cwl", bufs=2) as p_cwl, \
             tc.tile_pool(name="p_eoag", bufs=B_EOAG) as p_eoag, \
             tc.tile_pool(name="p_eo", bufs=2) as p_eo, \
             tc.tile_pool(name="p_eo8", bufs=2) as p_eo8, \
             tc.tile_pool(name="p_out", bufs=1) as p_out, \
             tc.tile_pool(name="p_warm", bufs=1) as p_warm, \
             tc.tile_pool(name="psum", bufs=1, space="PSUM") as p_ps:

            def mm_pair(psrow, lhsT_ap, rhs_of_j, start, stop, perf_mode=None):
                """Two matmuls sharing one stationary operand: the second
                skips its LDWEIGHTS and is order-pinned after the first."""
                prev = None
                for j in range(NJ):
                    m = nc.tensor.matmul(psrow[j], lhsT_ap, rhs_of_j(j),
                                         start=start, stop=stop,
                                         perf_mode=perf_mode)
                    if j > 0 and SKIP_LDW:
                        m.ins.ldweights = False
                        add_dep_helper(m.ins, prev.ins, False, "weight-reuse pair order")
                    prev = m

            def psum_tiles(tagp):
                return [[p_ps.tile([128, NFREE], dt.float32,
                                   name=f"{tagp}_{i}_{j}", tag=f"ps_{i}_{j}")
                         for j in range(NJ)] for i in range(4)]

            # ------------- Warmup: release the PE HAM clock gate -------------
            if N_WARM:
                warm_t = p_warm.tile([128, 128 + NFREE], io_dt)
                nc.vector.memset(warm_t, 0)
                warm_ps = p_ps.tile([128, NFREE], dt.float32,
                                    name="warm_ps", tag="ps_0_0")
                for _ in range(N_WARM):
                    nc.tensor.matmul(warm_ps, warm_t[:, :128],
                                     warm_t[:, 128:128 + NFREE],
                                     start=True, stop=True)

            ri_tiles, rw_tiles, cw_tiles = {}, {}, {}
            unflat = lambda t, n: t.rearrange("p (n d) -> p n d", n=n)

            # 2-block granule loads: each partition's data for consecutive
            # blocks is contiguous in the swizzled DRAM layouts, so pairing
            # blocks halves the descriptor count on the rings
            def load_ri(blk):
                b0 = blk - blk % 2
                t = p_ri.tile([128, 2 * SUB * S_LOC], s3_dt, tag="ri", name=f"ri_{b0}")
                nc.sync.dma_start(
                    out=t, in_=riT[:, b0 * SUB * S_LOC:(b0 + 2) * SUB * S_LOC])
                v = unflat(t, 2 * SUB)
                ri_tiles[b0] = v[:, :SUB]
                ri_tiles[b0 + 1] = v[:, SUB:]

            def load_rw(blk, q=None):
                b0 = blk - blk % 2
                t = p_rw.tile([128, 2 * SUB * D_OUT], s3_dt, tag="rw", name=f"rw_{b0}")
                d = (q or nc.scalar).dma_start(
                    out=t, in_=rw[:, b0 * SUB * D_OUT:(b0 + 2) * SUB * D_OUT])
                v = unflat(t, 2 * SUB)
                rw_tiles[b0] = v[:, :SUB]
                rw_tiles[b0 + 1] = v[:, SUB:]
                return d

            def load_cw(blk):
                b0 = blk - blk % 2
                t = p_cw.tile([128, 2 * SUB * S_LOC], dt.float8e4, tag="cw",
                              name=f"cw8_{b0}")
                nc.sync.dma_start(
                    out=t, in_=cwT8[:, b0 * SUB * S_LOC:(b0 + 2) * SUB * S_LOC])
                v = unflat(t, 2 * SUB)
                cw_tiles[b0] = v[:, :SUB]
                cw_tiles[b0 + 1] = v[:, SUB:]

            # ------------- Stage 1: eo_e = fT.T @ we, by c-halves ------------
            last_we = [None]
            last_copy = {"dve": None, "act": None}
            eo16 = {}
            s1_tiles = {}

            def load_s1(ch, blk):
                if (ch, blk) in s1_tiles:
                    return s1_tiles[(ch, blk)]
                f0 = (ch * S1BLK + blk) * (S1SUB * CH)
                w0_ = blk * (S1SUB * D_OUT)
                ft_f = p_ft.tile([128, S1SUB * CH], io_dt, tag="ft", name=f"ft_{ch}_{blk}")
                we_f = p_we.tile([128, S1SUB * D_OUT], io_dt, tag="we", name=f"we_{ch}_{blk}")
                if ch == 0 and blk == 0:
                    # the first block gates the whole pipeline: balance its
                    # 1.5 MB across the two fast HWDGE rings (~6.5 us each
                    # at ~116 GB/s).  Keep it OFF the gpsimd SWDGE ring
                    # (~25 GB/s — routing a 256 KB ft half there measured
                    # first-matmul at ~23 us instead of ~13).
                    nc.sync.dma_start(out=ft_f, in_=fT[:, f0:f0 + S1SUB * CH])
                    last_we[0] = nc.scalar.dma_start(
                        out=we_f[0:96, :],
                        in_=we[0:96, w0_:w0_ + S1SUB * D_OUT])
                    nc.sync.dma_start(
                        out=we_f[96:128, :],
                        in_=we[96:128, w0_:w0_ + S1SUB * D_OUT])
                else:
                    # balance the two HWDGE rings: the we stream alone is
                    # ~8.6us/tile (1 MB at ~116 GB/s) x 16 = 137us, which
                    # binds stage 1 on one ring; alternating ft/we between
                    # the rings puts ~103us on each
                    qa, qb = ((nc.sync, nc.scalar) if blk % 2 == ch
                              else (nc.scalar, nc.sync))
                    qa.dma_start(out=ft_f, in_=fT[:, f0:f0 + S1SUB * CH])
                    last_we[0] = qb.dma_start(
                        out=we_f, in_=we[:, w0_:w0_ + S1SUB * D_OUT])
                s1_tiles[(ch, blk)] = (unflat(ft_f, S1SUB), unflat(we_f, S1SUB))
                return s1_tiles[(ch, blk)]

            for ch in range(2):
                psums = psum_tiles(f"s1h{ch}")
                for blk in range(S1BLK):
                    # pre-issue the FIRST stage-3/2 operand pairs late in
                    # half 0, one per block (early half-0 slots would steal
                    # ring time from the ft/we ramp; half-1 slots dilute the
                    # boundary supply; later pairs arrive in time via the
                    # stage-2/3 emission path)
                    if ch == 0 and S1BLK - 1 - 3 * NPRE <= blk < S1BLK - 1:
                        k, r = divmod(S1BLK - 1 - blk - 1, 3)
                        if k < NPRE:
                            # 2-block granules: k-th pre-issue covers blocks
                            # 2k, 2k+1 of each operand
                            (load_ri,
                             lambda b: load_rw(b, q=nc.scalar),
                             load_cw)[r](2 * k)
                    # (hoisting ch1-blk0/1 loads into half 0 measured WORSE:
                    # the rings run at ~1.3x of stage-1's consumption, so a
                    # 3 MB insertion starves half-0's own tail for ~8 us)
                    ft_t, we_t = load_s1(ch, blk)
                    if blk < S1BLK - 1:
                        for sub in range(S1SUB):
                            kt = blk * S1SUB + sub
                            for i in range(4):
                                mm_pair(psums[i],
                                        ft_t[:, sub, i * 128:(i + 1) * 128],
                                        lambda j, sub=sub: we_t[:, sub, j * NFREE:(j + 1) * NFREE],
                                        start=(kt == 0), stop=False)
                eo8_half = p_eo8.tile([128, 4 * D_OUT], dt.float8e4, tag="eo8",
                                      name=f"eo8_{ch}")
                if LOCAL_BF16:
                    eo_half = p_eo.tile([128, 4 * D_OUT], io_dt, tag="eo",
                                        name=f"eo_{ch}")
                    eo16[ch] = unflat(eo_half, 4)
                # last block: finish each PSUM bank in turn; copy to bf16
                # (DVE, kept local) and scaled fp8 (ACT, AllGathered)
                for i in range(4):
                    for j in range(NJ):
                        for sub in range(S1SUB):
                            nc.tensor.matmul(
                                psums[i][j],
                                ft_t[:, sub, i * 128:(i + 1) * 128],
                                we_t[:, sub, j * NFREE:(j + 1) * NFREE],
                                start=False, stop=(sub == S1SUB - 1))
                        o0 = i * D_OUT + j * NFREE
                        if LOCAL_BF16:
                            nc.vector.tensor_copy(
                                out=eo_half[:, o0:o0 + NFREE], in_=psums[i][j])
                        # split the fp8 scaled copies across DVE and ACT so the
                        # 8-bank turnaround doesn't serialize on one engine
                        # (single-engine measured a 4.4us PE stall + HAM
                        # re-throttle at the half boundary)
                        if SPLIT_COPY and j % 2 == 0 and not LOCAL_BF16:
                            last_copy["dve"] = nc.vector.tensor_scalar_mul(
                                eo8_half[:, o0:o0 + NFREE], psums[i][j],
                                1.0 / S2_SCALE)
                        else:
                            last_copy["act"] = nc.scalar.mul(
                                eo8_half[:, o0:o0 + NFREE], psums[i][j],
                                1.0 / S2_SCALE)
                    # SWDGE queue keeps these late-gated writes out of the
                    # HWDGE FIFOs (head-of-line blocking of operand loads)
                    if i % 2:
                        nc.gpsimd.dma_start(
                            out=ag_in[ch][:, (i - 1) * D_OUT:(i + 1) * D_OUT],
                            in_=eo8_half[:, (i - 1) * D_OUT:(i + 1) * D_OUT])
                # chunked fp8 AllGather (half the bytes of the bf16 one)
                nc.gpsimd.collective_compute(
                    "AllGather", mybir.AluOpType.bypass,
                    replica_groups=[list(range(N_CORES))],
                    ins=[ag_in[ch][:].opt()], outs=[ag_out[ch][:].opt()])

            # ------------- Stages 2+3, interleaved ---------------------------
            # Emission order: s3 blocks 0-5 (covers AG0 tail), local-bf16 twin
            # of half 0, s2 fp8 blocks 0-7, s3 blocks 6-7 (covers AG1 tail),
            # local-bf16 twin of half 1, s2 fp8 blocks 8-15 (block 15
            # finishes: PSUM->SBUF copies + output DMA).
            psums = psum_tiles("s23")
            s3_dr = s3_dt == dt.float8e4
            out_sb = p_out.tile([128, 4, D_OUT], dt.float32)
            eoag_state = {"prev": None}
            S3_SPLIT = 6

            def s2_local_block(h):
                # this core's own eo chunk, bf16, no collective dependency
                t = p_cwl.tile([128, SUB * S_LOC], io_dt, tag="cwl", name=f"cw16_{h}")
                nc.sync.dma_start(
                    out=t, in_=cwT16[:, h * SUB * S_LOC:(h + 1) * SUB * S_LOC])
                cw_t = unflat(t, SUB)
                for sub in range(SUB):
                    for i in range(4):
                        mm_pair(psums[i],
                                cw_t[:, sub, i * 128:(i + 1) * 128],
                                lambda j, sub=sub: eo16[h][:, sub, j * NFREE:(j + 1) * NFREE],
                                start=False, stop=False)

            def load_eoag(blk):
                half = blk // 8              # ag chunk this block reads
                rk = blk % 8                 # rank whose eo chunk this is
                if blk not in cw_tiles:
                    load_cw(blk)
                eo_f = p_eoag.tile([128, SUB * D_OUT], dt.float8e4, tag="eoag",
                                   name=f"eoag_{blk}")
                # alternate rings: stage-1's ft/we streams are done by now, so
                # the sync ring is free to carry half the eoag supply
                q = nc.scalar if blk % 2 else nc.sync
                eoag_dma = q.dma_start(
                    out=eo_f, in_=ag_out[half][rk * 128:(rk + 1) * 128, :])
                # pin each queue's first eoag after stage-1's LAST PSUM->SBUF
                # copies and the operand loads, then keep block order within
                # the queue.  Without this the scheduler hoists an AG-gated
                # dispatch ahead of the half-1 copies: the engine FIFO
                # head-of-line blocks on the collective, PSUM turnaround
                # stalls, and every core's stage-1 tail (so the collective
                # itself) serializes (measured 68us PE stall).
                key = "prev_s" if blk % 2 else "prev_y"
                prev = eoag_state.get(key)
                if prev is not None:
                    add_dep_helper(eoag_dma.ins, prev.ins, False,
                                   "eoag in block order per queue")
                else:
                    for root in (last_copy["dve"], last_copy["act"],
                                 eoag_state.get("root") or last_we[0]):
                        if root is not None:
                            add_dep_helper(eoag_dma.ins, root.ins, False,
                                           "first eoag after stage-1 tail")
                eoag_state[key] = eoag_dma
                return unflat(eo_f, SUB)

            def s2_block(blk):
                eo_t = load_eoag(blk)
                cw_t = cw_tiles[blk]
                for s2i in range(SUB // 2):
                    for i in range(4):
                        mm_pair(psums[i],
                                cw_t[:, 2 * s2i:2 * s2i + 2, i * 128:(i + 1) * 128],
                                lambda j, s2i=s2i: eo_t[:, 2 * s2i:2 * s2i + 2,
                                                        j * NFREE:(j + 1) * NFREE],
                                start=False, stop=False,
                                perf_mode=mybir.MatmulPerfMode.DoubleRow)
                return cw_t, eo_t

            def s2_tail(blks):
                # bank-major sweep over the last blocks: each (i, j0/j1) bank
                # pair finishes ~4.2 us apart, so its PSUM->SBUF copies (split
                # DVE/ACT) and output DMAs (split sync/scalar) hide behind the
                # remaining matmuls instead of stacking up after the last one
                tiles = []
                for b in blks:
                    eo_t = load_eoag(b)
                    tiles.append((cw_tiles[b], eo_t))
                for i in range(4):
                    for bi, (cw_t, eo_t) in enumerate(tiles):
                        for s2i in range(SUB // 2):
                            mm_pair(psums[i],
                                    cw_t[:, 2 * s2i:2 * s2i + 2, i * 128:(i + 1) * 128],
                                    lambda j, s2i=s2i, eo_t=eo_t: eo_t[:, 2 * s2i:2 * s2i + 2,
                                                                      j * NFREE:(j + 1) * NFREE],
                                    start=False,
                                    stop=(bi == len(tiles) - 1 and s2i == SUB // 2 - 1),
                                    perf_mode=mybir.MatmulPerfMode.DoubleRow)
                    for j in range(NJ):
                        if SPLIT_COPY and j % 2:
                            nc.scalar.copy(out_sb[:, i, j * NFREE:(j + 1) * NFREE],
                                           psums[i][j])
                        else:
                            nc.vector.tensor_copy(
                                out=out_sb[:, i, j * NFREE:(j + 1) * NFREE],
                                in_=psums[i][j])
                        q = nc.scalar if (SPLIT_COPY and j % 2) else nc.sync
                        q.dma_start(
                            out=out[i * 128:(i + 1) * 128, j * NFREE:(j + 1) * NFREE]
                                .rearrange("(n p) d -> p n d", p=128),
                            in_=out_sb[:, i:i + 1, j * NFREE:(j + 1) * NFREE])

            def s3_blocks(b0, b1, start):
                if b0 == 0:
                    # issue the remaining stage-3 operand loads now, BEFORE any
                    # AG-gated eoag dispatch enters the rings (ring descriptors
                    # process in order; anything queued behind a gated eoag
                    # waits for the collective)
                    for blk in range(NBLK):
                        if blk not in ri_tiles:
                            load_ri(blk)
                        if blk not in rw_tiles:
                            eoag_state["root"] = load_rw(blk)
                for blk in range(b0, b1):
                    if blk not in ri_tiles:
                        load_ri(blk)
                    ri_t = ri_tiles[blk]
                    if blk not in rw_tiles:
                        eoag_state["root"] = load_rw(blk)
                    rw_t = rw_tiles[blk]
                    if s3_dr:
                        for s2 in range(SUB // 2):
                            for i in range(4):
                                mm_pair(psums[i],
                                        ri_t[:, 2 * s2:2 * s2 + 2, i * 128:(i + 1) * 128],
                                        lambda j, s2=s2: rw_t[:, 2 * s2:2 * s2 + 2,
                                                              j * NFREE:(j + 1) * NFREE],
                                        start=(start and blk == b0 and s2 == 0),
                                        stop=False,
                                        perf_mode=mybir.MatmulPerfMode.DoubleRow)
                    else:
                        for sub in range(SUB):
                            for i in range(4):
                                mm_pair(psums[i],
                                        ri_t[:, sub, i * 128:(i + 1) * 128],
                                        lambda j, sub=sub: rw_t[:, sub, j * NFREE:(j + 1) * NFREE],
                                        start=(start and blk == b0 and sub == 0),
                                        stop=False)

            s3_blocks(0, S3_SPLIT, start=True)       # covers AG0's tail
            if LOCAL_BF16:
                s2_local_block(0)                    # AG-independent filler
            for blk in range(8):                     # stage-2 fp8 half 0
                s2_block(blk)
            s3_blocks(S3_SPLIT, NBLK, start=False)   # covers AG1's tail
            if LOCAL_BF16 == 2:
                s2_local_block(1)                    # AG-independent filler
            for blk in range(8, 10):                 # stage-2 fp8 half 1
                s2_block(blk)
            s2_tail(list(range(10, ECT // SUB)))     # bank-major finish

    nc.finalize()
    if PAIR_SKIP:
        d, n_del = _pairskip_ldweights_json(json.loads(nc.to_json_bytes()))
        fused = json.dumps(d).encode()
        nc.to_json_bytes = lambda: fused
    return nc


def _get_prog(ldw_opt):
    key = (ldw_opt,)
    if key not in _prog_cache:
        if ldw_opt:
            _patch_ldw_opt()
        _prog_cache[key] = _build(ldw_opt)
    return _prog_cache[key]


def _swz(a, nblk, nsub):
    """[nblk*nsub*128, d] contraction-major -> partition-major [128, nblk*nsub*d]
    with each partition's data contiguous (fat DMA descriptors)."""
    d = a.shape[1]
    return np.ascontiguousarray(
        a.reshape(nblk, nsub, 128, d).transpose(2, 0, 1, 3).reshape(128, nblk * nsub * d))


def _prep_in_maps(inputs, expert_w, residual_w, combine_weights, residual_weight):
    np_dt = BF16
    front = inputs[:E * C].reshape(E, C, D_IN)
    resid = inputs[E * C:]                       # [TOK, D_IN]
    rwt = residual_weight.reshape(TOK, 2)
    w0, w1 = rwt[:, 0], rwt[:, 1]

    s3_np = F8 if FP8_S3 else np_dt
    rw_scale = FP8_S if FP8_S3 else 1.0
    rw_sw = _swz((residual_w * rw_scale).astype(s3_np), 8, 4)            # [128, 8*4*1024]
    resid_s = resid * (w1[:, None] / rw_scale)   # fold w1 and 1/S (fp32)
    in_maps = []
    for r in range(N_CORES):
        sl = slice(r * S_LOC, (r + 1) * S_LOC)
        fT = front[r].T.astype(np_dt)                                    # [D_IN, C]
        # fT layout [128, (ch, blk, sub, c)]: the kernel reads c-halves
        fT_sw = np.ascontiguousarray(
            fT.reshape(16, 2, 128, 2, CH).transpose(2, 3, 0, 1, 4).reshape(128, -1))
        we_sw = _swz(expert_w[r].astype(np_dt), 16, 2)                   # [128, 16*2*1024]
        # centered combine weights: v = (cw - 0.5) * w0; the removed mean is
        # added back exactly on the host (rank-1 term, see _run)
        cw_c = (combine_weights[sl] - 0.5) * w0[sl, None, None]          # [S_LOC, E, C]
        # contraction rows ordered (c-half chunk, expert, c-within-half) to
        # match the chunked AllGather's concatenation
        cwT_f32 = (cw_c.reshape(S_LOC, E, 2, CH).transpose(2, 1, 3, 0)
                   .reshape(E * C, S_LOC))
        cw8_src = cwT_f32 * S2_SCALE
        if LOCAL_BF16:
            # this core's own chunks run through the local bf16 path; zero
            # their fp8 slots (the SPMD program still executes them)
            loc_blocks = [r, 8 + r][:LOCAL_BF16]
            cw8_src = cw8_src.copy()
            for b in loc_blocks:
                cw8_src[b * 512:(b + 1) * 512] = 0.0
            cw16 = np.concatenate(
                [cwT_f32[b * 512:(b + 1) * 512]
                 for b in sorted(loc_blocks)], axis=0)
            cwT16_sw = _swz(cw16.astype(np_dt), LOCAL_BF16, 4)
        cwT8_sw = _swz(cw8_src.astype(F8), 16, 4)                        # [128, 16*4*512]
        riT_sw = _swz(resid_s[sl].T.astype(s3_np), 8, 4)                 # [128, 8*4*512]
        m = {"fT": fT_sw, "we": we_sw, "cwT8": cwT8_sw, "riT": riT_sw, "rw": rw_sw}
        if LOCAL_BF16:
            m["cwT16"] = cwT16_sw
        in_maps.append(m)
    return in_maps


def _run(inputs, expert_w, expert_b, residual_w, residual_b,
         combine_weights, residual_weight, ldw_opt=None, trace=False, mode=None):
    import jax
    try:
        if jax.config.jax_compilation_cache_dir is None:
            jax.config.update("jax_compilation_cache_dir", "/tmp/jax_cache_trn_moe")
            jax.config.update("jax_persistent_cache_min_compile_time_secs", 0.5)
    except Exception:
        pass
    from concourse.bass_utils import run_bass_kernel_spmd

    ldw_opt = LDW_OPT if ldw_opt is None else ldw_opt
    inputs = np.asarray(inputs, dtype=np.float32)
    expert_w = np.asarray(expert_w, dtype=np.float32)
    expert_b = np.asarray(expert_b, dtype=np.float32)
    residual_w = np.asarray(residual_w, dtype=np.float32)
    residual_b = np.asarray(residual_b, dtype=np.float32)
    combine_weights = np.asarray(combine_weights, dtype=np.float32)
    residual_weight = np.asarray(residual_weight, dtype=np.float32)

    nc = _get_prog(ldw_opt)
    in_maps = _prep_in_maps(inputs, expert_w, residual_w, combine_weights,
                            residual_weight)
    res = run_bass_kernel_spmd(nc, in_maps, list(range(N_CORES)), trace=trace)
    out = np.concatenate([res.results[r]["out"] for r in range(N_CORES)], axis=0)

    rwt = residual_weight.reshape(TOK, 2)
    # exact centering term: 0.5 * w0[s] * colsum(front @ W)[m], float64
    front = inputs[:E * C].reshape(E, C, D_IN)
    G = np.zeros(D_OUT, np.float64)
    for e in range(E):
        G += front[e].sum(axis=0).astype(np.float64) @ expert_w[e].astype(np.float64)
    out = out + (0.5 * rwt[:, 0:1]) * G[None, :].astype(np.float32)
    # exact bias contributions (zero in practice, but keep the math honest)
    if residual_b.any():
        out = out + rwt[:, 1:2] * residual_b[None, :]
    if expert_b.any():
        cs = combine_weights.sum(axis=2)                    # [TOK, E]
        out = out + rwt[:, 0:1] * (cs @ expert_b)
    return out.reshape(B, S, D_OUT).astype(np.float32), res


def kernel(**kw):
    out, _ = _run(**kw)
    return out


# revision 40
# speedup vs baseline: 1.0402x; 1.0381x over previous
"""Expert-parallel MoE "behind" block + residual on 8 Trainium2 NeuronCores.

Reference computation (fp32):
    front      = inputs[:E*C].reshape(E, C, D_IN)
    expert_out = einsum("ecd,edm->ecm", front, expert_w) + expert_b
    combined   = einsum("sec,ecm->sm", combine_weights, expert_out)
    resid      = inputs[E*C:] @ residual_w + residual_b
    out        = combined * w0[:, None] + resid * w1[:, None]

Sharding (8 cores):
  Stage 1 (expert-parallel, bf16): core e computes eo_e = front_e @ W_e
  [C, D_OUT] in two c-halves; each half is copied out of PSUM twice — once
  to bf16 (kept locally) and once to fp8 e4m3 with a 1/4 scale — and the
  fp8 copy is AllGathered (halved collective payload vs bf16: ~38 us/chunk).
  Stage 3 (token-parallel residual): fp8 e4m3 DoubleRow as before.
  Stage 2 (token-parallel combine): ALL 16 k-blocks run fp8 e4m3 DoubleRow
  (2 k-rows/cycle) against the AllGathered fp8 eo.  Two accuracy devices
  make this fit the 2e-2 rel-l2 bar (measured 1.664e-2 baseline budget):

  * Mean-centering: cw is uniform[0,1); the device matmuls use
    v = (cw - 0.5)*w0 (rms halved), which halves BOTH fp8 error terms
    (q(v)·eo and v·q(eo)); the removed mean is an exact host-side rank-1
    term  0.5*w0[s] * colsum(eo)[m]  with colsum(eo) = sum_e
    (colsum front_e) @ W_e computed on host in float64 (67 MFLOP, same
    spirit as the existing exact expert_b/residual_b foldbacks).
  * Local-bf16 twins: each core's OWN two eo chunks (half h, rank r) never
    need the collective — they are read from the local bf16 eo_half and
    contracted in bf16.  SPMD runs one program on all cores, so the fp8
    slots for those two blocks still execute but with their cw data zeroed
    host-side (exact zero contribution), and two extra bf16 blocks carry
    the real values.  Costs 2x4.2 us of zero-matmuls; removes the two
    largest error terms per core.

  Default config runs ALL 16 blocks fp8 (LOCAL_BF16=0): measured rel-l2
  1.8876e-2 on HW, bit-identical to the numpy emulation and deterministic
  across runs (fixed seed-0 inputs; the metric averages 4M outputs, so it
  is also insensitive to input resampling).  LOCAL_BF16=2 gives 1.767e-2
  at +25 us.

All DRAM operands are host-swizzled partition-major as before (fat DMA
descriptors).  Perf notes (measured on these trn2 cores):
  * Every 512-col MM cadences at exactly 263 ns (512 cycles at the
    sustained 13/16-throttled 1.95 GHz clock), bf16 and fp8-DR alike; the
    kernel is cadence-bound: 894 MMs = 235 us + ~12 us boot head + ~2 us
    gaps + ~2 us tail = 252-262 us (vs 316-326 us baseline).  Some runs
    draw the full 2.4 GHz clock (no SW throttle) — the ring balancing
    below is what lets those run supply-clean.
  * Stage-1 ft/we tile loads alternate between the two HWDGE rings per
    block: the we stream alone is ~8.6 us/MB-tile x 16 = 137 us, which
    out-runs stage-1's PE time on one ring.  S1SUB=4 (4 k-subtiles per
    DMA) halves descriptor counts; ri/rw/cw ship as 2-block granules.
  * eoag loads alternate sync/scalar, with each queue's FIRST eoag pinned
    after stage-1's last PSUM copies — an AG-gated dispatch hoisted ahead
    of them head-of-line blocks the engine FIFO, stalls PSUM turnaround,
    and serializes the collective across all cores (measured 68 us).
  * The two fp8 AllGathers run serially on the CC engine (~100 us
    combined); stage-3 splits 6+2 around stage-2 half 0 to cover their
    tails.
  * The last 6 combine blocks run bank-major so each PSUM bank's output
    copy + DMA hides behind the remaining matmuls.
  * 16 warmup matmuls release the PE HAM clock gate during the ~12 us
    boot+DMA head (N_WARM=0 measured worse: mid-stage-1 HAM wobble).
  * _pairskip drops redundant Ldweights.

Env knobs: TRN_LOCAL_BF16=0 (default; 2 = bf16 twins, lower error),
TRN_FP8_S3=1, TRN_WARM=16, TRN_SKIP_LDW=1, TRN_PAIR_SKIP=1.
"""

import json
import os
import numpy as np
import ml_dtypes

E, C, D_IN, D_OUT = 8, 1024, 4096, 1024
B, S = 2, 2048
TOK = B * S                 # 4096 tokens
N_CORES = 8
S_LOC = TOK // N_CORES      # 512 tokens per core
CH = C // 2                 # c-half = 512
BF16 = ml_dtypes.bfloat16
F8 = ml_dtypes.float8_e4m3

LDW_OPT = os.environ.get("TRN_LDW_OPT", "0") == "1"
SKIP_LDW = os.environ.get("TRN_SKIP_LDW", "1") == "1"
PAIR_SKIP = os.environ.get("TRN_PAIR_SKIP", "1") == "1"
FP8_S3 = os.environ.get("TRN_FP8_S3", "1") == "1"
N_WARM = int(os.environ.get("TRN_WARM", "28"))
SPLIT_COPY = os.environ.get("TRN_SPLIT_COPY", "1") == "1"
# 2 = each core contracts its own two eo chunks in bf16 (fp8 slots zeroed);
# 0 = all 16 combine blocks in fp8 (rel-l2 1.888e-2 vs 1.767e-2, both
# deterministic on the fixed seed-0 inputs; 0 saves ~25 us of PE time)
LOCAL_BF16 = int(os.environ.get("TRN_LOCAL_BF16", "0"))
# fp8 e4m3 operand scalings (products exact):
FP8_S = 4.0                 # stage 3: riT*(1/S), rw*S
S2_SCALE = 4.0              # stage 2: cw8*S, eo8*(1/S)

_prog_cache = {}


def _pairskip_ldweights_json(d):
    """Delete Ldweights that reload the weights already in the PE array.

    bass unconditionally emits Ldweights+Matmult for every matmul, so the
    mm_pair weight-reuse never took effect.  Measured on HW: a 512-row bf16
    MM whose weights must load first has a 263 ns cadence vs 216 ns when the
    array already holds them.  Emulate the array state over the PE queue and
    drop any Ldweights whose weights AP exactly matches the one currently
    loaded; its semaphore wait (rare) moves to the next PE instruction or an
    EventSemaphore.
    """
    n_del = 0
    for fn in d["functions"]:
        for blk in fn["blocks"]:
            out, loaded, carry_wait = [], None, []
            for ins in blk["instructions"]:
                op = ins["opcode"]
                if op == "Ldweights":
                    sig = json.dumps(ins["ins"][0], sort_keys=True)
                    if sig == loaded:
                        w = (ins.get("sync_info") or {}).get("on_wait") or []
                        carry_wait.extend(w)
                        n_del += 1
                        continue
                    loaded = sig
                elif op == "Matmult" and carry_wait:
                    si = ins.get("sync_info") or {"on_update": [], "on_wait": []}
                    ins["sync_info"] = si
                    if not (si.get("on_wait") or []):
                        si["on_wait"] = [carry_wait.pop(0)]
                    for w in carry_wait:
                        out.append({
                            "debug": ins.get("debug", 0), "engine": "PE",
                            "ins": [], "outs": [],
                            "name": ins["name"] + f"_cw{len(out)}",
                            "opcode": "EventSemaphore",
                            "sync_info": {"on_update": [], "on_wait": [w]},
                        })
                    carry_wait = []
                out.append(ins)
            assert not carry_wait
            blk["instructions"] = out
    return d, n_del


def _patch_ldw_opt():
    from concourse import bass_utils
    if getattr(bass_utils, "_ldw_opt_patched", False):
        return
    orig = bass_utils.run_command

    def patched(argv, **kw):
        argv = ["--enable-ldw-opt=true" if a == "--enable-ldw-opt=false" else a
                for a in argv]
        return orig(argv, **kw)

    bass_utils.run_command = patched
    bass_utils._ldw_opt_patched = True


def _build(ldw_opt):
    import concourse.bass as bass  # noqa: F401
    import concourse.mybir as mybir
    from concourse import bacc
    from concourse.tile import TileContext, add_dep_helper

    dt = mybir.dt
    io_dt = dt.bfloat16

    nc = bacc.Bacc("TRN2", target_bir_lowering=False, debug=False, num_devices=N_CORES)

    s3_dt = dt.float8e4 if FP8_S3 else io_dt
    fT = nc.declare_dram_parameter("fT", [128, D_IN // 128 * C], io_dt, isOutput=False)
    we = nc.declare_dram_parameter("we", [128, D_IN // 128 * D_OUT], io_dt, isOutput=False)
    cwT8 = nc.declare_dram_parameter("cwT8", [128, E * C // 128 * S_LOC], dt.float8e4,
                                     isOutput=False)
    cwT16 = (nc.declare_dram_parameter("cwT16", [128, LOCAL_BF16 * 4 * S_LOC], io_dt,
                                       isOutput=False) if LOCAL_BF16 else None)
    riT = nc.declare_dram_parameter("riT", [128, D_IN // 128 * S_LOC], s3_dt, isOutput=False)
    rw = nc.declare_dram_parameter("rw", [128, D_IN // 128 * D_OUT], s3_dt, isOutput=False)
    out = nc.declare_dram_parameter("out", [S_LOC, D_OUT], dt.float32, isOutput=True)

    # variant tag so differently-compiled builds never share a jax cache entry
    nc.dram_tensor(
        f"variant_v2_{int(ldw_opt)}_{int(SKIP_LDW)}_{int(PAIR_SKIP)}"
        f"_{int(FP8_S3)}_{N_WARM}_{int(SPLIT_COPY)}_loc{LOCAL_BF16}",
        [1, 1], dt.float32)

    ag_in = [nc.dram_tensor(f"ag_in{h}", [128, 4 * D_OUT], dt.float8e4) for h in range(2)]
    ag_out = [nc.dram_tensor(f"ag_out{h}", [N_CORES * 128, 4 * D_OUT], dt.float8e4,
                             addr_space="Shared") for h in range(2)]

    KT = D_IN // 128            # 32 contraction tiles
    SUB = 4                     # k-subtiles per DMA'd block
    NBLK = KT // SUB            # 8 stage-3 blocks
    ECT = (E * C) // 128        # 64 combine contraction tiles
    NFREE = 512                 # PSUM bank cap: 512 fp32 out elements = 2 KiB
    NJ = D_OUT // NFREE

    S1SUB = 4                   # k-subtiles per stage-1 DMA'd block: 4KB/8KB
    S1BLK = KT // S1SUB         # descriptors halve ring time vs 2-subtile
    B_FT, B_WE, B_RI, B_RW, B_CW, B_EOAG = 7, 6, 4, 4, 3, 8
    NPRE = 1                    # stage-3/2 block-pairs prefetched during stage 1
    with TileContext(nc) as tc:
        with tc.tile_pool(name="p_ft", bufs=B_FT) as p_ft, \
             tc.tile_pool(name="p_we", bufs=B_WE) as p_we, \
             tc.tile_pool(name="p_ri", bufs=B_RI) as p_ri, \
             tc.tile_pool(name="p_rw", bufs=B_RW) as p_rw, \
             tc.tile_pool(name="p_cw", bufs=B_CW) as p_cw, \
             tc.tile_pool(name="p_cwl", bufs=2) as p_cwl, \
             tc.tile_pool(name="p_eoag", bufs=B_EOAG) as p_eoag, \
             tc.tile_pool(name="p_eo", bufs=2) as p_eo, \
             tc.tile_pool(name="p_eo8", bufs=2) as p_eo8, \
             tc.tile_pool(name="p_out", bufs=1) as p_out, \
             tc.tile_pool(name="p_warm", bufs=1) as p_warm, \
             tc.tile_pool(name="psum", bufs=1, space="PSUM") as p_ps:

            def mm_pair(psrow, lhsT_ap, rhs_of_j, start, stop, perf_mode=None):
                """Two matmuls sharing one stationary operand: the second
                skips its LDWEIGHTS and is order-pinned after the first."""
                prev = None
                for j in range(NJ):
                    m = nc.tensor.matmul(psrow[j], lhsT_ap, rhs_of_j(j),
                                         start=start, stop=stop,
                                         perf_mode=perf_mode)
                    if j > 0 and SKIP_LDW:
                        m.ins.ldweights = False
                        add_dep_helper(m.ins, prev.ins, False, "weight-reuse pair order")
                    prev = m

            def psum_tiles(tagp):
                return [[p_ps.tile([128, NFREE], dt.float32,
                                   name=f"{tagp}_{i}_{j}", tag=f"ps_{i}_{j}")
                         for j in range(NJ)] for i in range(4)]

            # ------------- Warmup: release the PE HAM clock gate -------------
            if N_WARM:
                warm_t = p_warm.tile([128, 128 + NFREE], io_dt)
                nc.vector.memset(warm_t, 0)
                warm_ps = p_ps.tile([128, NFREE], dt.float32,
                                    name="warm_ps", tag="ps_0_0")
                for _ in range(N_WARM):
                    nc.tensor.matmul(warm_ps, warm_t[:, :128],
                                     warm_t[:, 128:128 + NFREE],
                                     start=True, stop=True)

            ri_tiles, rw_tiles, cw_tiles = {}, {}, {}
            unflat = lambda t, n: t.rearrange("p (n d) -> p n d", n=n)

            # 2-block granule loads: each partition's data for consecutive
            # blocks is contiguous in the swizzled DRAM layouts, so pairing
            # blocks halves the descriptor count on the rings
            def load_ri(blk):
                b0 = blk - blk % 2
                t = p_ri.tile([128, 2 * SUB * S_LOC], s3_dt, tag="ri", name=f"ri_{b0}")
                nc.sync.dma_start(
                    out=t, in_=riT[:, b0 * SUB * S_LOC:(b0 + 2) * SUB * S_LOC])
                v = unflat(t, 2 * SUB)
                ri_tiles[b0] = v[:, :SUB]
                ri_tiles[b0 + 1] = v[:, SUB:]

            def load_rw(blk, q=None):
                b0 = blk - blk % 2
                t = p_rw.tile([128, 2 * SUB * D_OUT], s3_dt, tag="rw", name=f"rw_{b0}")
                d = (q or nc.scalar).dma_start(
                    out=t, in_=rw[:, b0 * SUB * D_OUT:(b0 + 2) * SUB * D_OUT])
                v = unflat(t, 2 * SUB)
                rw_tiles[b0] = v[:, :SUB]
                rw_tiles[b0 + 1] = v[:, SUB:]
                return d

            def load_cw(blk):
                b0 = blk - blk % 2
                t = p_cw.tile([128, 2 * SUB * S_LOC], dt.float8e4, tag="cw",
                              name=f"cw8_{b0}")
                nc.sync.dma_start(
                    out=t, in_=cwT8[:, b0 * SUB * S_LOC:(b0 + 2) * SUB * S_LOC])
                v = unflat(t, 2 * SUB)
                cw_tiles[b0] = v[:, :SUB]
                cw_tiles[b0 + 1] = v[:, SUB:]

            # ------------- Stage 1: eo_e = fT.T @ we, by c-halves ------------
            last_we = [None]
            last_copy = {"dve": None, "act": None}
            eo16 = {}
            s1_tiles = {}
            pending_ag = [None]

            def load_s1(ch, blk):
                if (ch, blk) in s1_tiles:
                    return s1_tiles[(ch, blk)]
                f0 = (ch * S1BLK + blk) * (S1SUB * CH)
                w0_ = blk * (S1SUB * D_OUT)
                ft_f = p_ft.tile([128, S1SUB * CH], io_dt, tag="ft", name=f"ft_{ch}_{blk}")
                we_f = p_we.tile([128, S1SUB * D_OUT], io_dt, tag="we", name=f"we_{ch}_{blk}")
                if ch == 0 and blk == 0:
                    # the first block gates the whole pipeline: balance its
                    # 1.5 MB across the two fast HWDGE rings (~6.5 us each
                    # at ~116 GB/s).  Keep it OFF the gpsimd SWDGE ring
                    # (~25 GB/s — routing a 256 KB ft half there measured
                    # first-matmul at ~23 us instead of ~13).
                    nc.sync.dma_start(out=ft_f, in_=fT[:, f0:f0 + S1SUB * CH])
                    last_we[0] = nc.scalar.dma_start(
                        out=we_f[0:96, :],
                        in_=we[0:96, w0_:w0_ + S1SUB * D_OUT])
                    nc.sync.dma_start(
                        out=we_f[96:128, :],
                        in_=we[96:128, w0_:w0_ + S1SUB * D_OUT])
                else:
                    # balance the two HWDGE rings: the we stream alone is
                    # ~8.6us/tile (1 MB at ~116 GB/s) x 16 = 137us, which
                    # binds stage 1 on one ring; alternating ft/we between
                    # the rings puts ~103us on each
                    qa, qb = ((nc.sync, nc.scalar) if blk % 2 == ch
                              else (nc.scalar, nc.sync))
                    qa.dma_start(out=ft_f, in_=fT[:, f0:f0 + S1SUB * CH])
                    last_we[0] = qb.dma_start(
                        out=we_f, in_=we[:, w0_:w0_ + S1SUB * D_OUT])
                s1_tiles[(ch, blk)] = (unflat(ft_f, S1SUB), unflat(we_f, S1SUB))
                return s1_tiles[(ch, blk)]

            for ch in range(2):
                psums = psum_tiles(f"s1h{ch}")
                for blk in range(S1BLK):
                    # pre-issue the FIRST stage-3/2 operand pairs late in
                    # half 0, one per block (early half-0 slots would steal
                    # ring time from the ft/we ramp; half-1 slots dilute the
                    # boundary supply; later pairs arrive in time via the
                    # stage-2/3 emission path)
                    if ch == 0 and S1BLK - 1 - 3 * NPRE <= blk < S1BLK - 1:
                        k, r = divmod(S1BLK - 1 - blk - 1, 3)
                        if k < NPRE:
                            # 2-block granules: k-th pre-issue covers blocks
                            # 2k, 2k+1 of each operand
                            (load_ri,
                             lambda b: load_rw(b, q=nc.scalar),
                             load_cw)[r](2 * k)
                    # (hoisting ch1-blk0/1 loads into half 0 measured WORSE:
                    # the rings run at ~1.3x of stage-1's consumption, so a
                    # 3 MB insertion starves half-0's own tail for ~8 us)
                    ft_t, we_t = load_s1(ch, blk)
                    if ch == 1 and blk == 1 and pending_ag[0]:
                        pending_ag[0]()
                        pending_ag[0] = None
                    if blk < S1BLK - 1:
                        for sub in range(S1SUB):
                            kt = blk * S1SUB + sub
                            for i in range(4):
                                mm_pair(psums[i],
                                        ft_t[:, sub, i * 128:(i + 1) * 128],
                                        lambda j, sub=sub: we_t[:, sub, j * NFREE:(j + 1) * NFREE],
                                        start=(kt == 0), stop=False)
                eo8_half = p_eo8.tile([128, 4 * D_OUT], dt.float8e4, tag="eo8",
                                      name=f"eo8_{ch}")
                if LOCAL_BF16:
                    eo_half = p_eo.tile([128, 4 * D_OUT], io_dt, tag="eo",
                                        name=f"eo_{ch}")
                    eo16[ch] = unflat(eo_half, 4)
                # last block: finish each PSUM bank in turn; copy to bf16
                # (DVE, kept local) and scaled fp8 (ACT, AllGathered)
                for i in range(4):
                    for j in range(NJ):
                        for sub in range(S1SUB):
                            nc.tensor.matmul(
                                psums[i][j],
                                ft_t[:, sub, i * 128:(i + 1) * 128],
                                we_t[:, sub, j * NFREE:(j + 1) * NFREE],
                                start=False, stop=(sub == S1SUB - 1))
                        o0 = i * D_OUT + j * NFREE
                        if LOCAL_BF16:
                            nc.vector.tensor_copy(
                                out=eo_half[:, o0:o0 + NFREE], in_=psums[i][j])
                        # split the fp8 scaled copies across DVE and ACT so the
                        # 8-bank turnaround doesn't serialize on one engine
                        # (single-engine measured a 4.4us PE stall + HAM
                        # re-throttle at the half boundary)
                        if SPLIT_COPY and j % 2 == 0 and not LOCAL_BF16:
                            last_copy["dve"] = nc.vector.tensor_scalar_mul(
                                eo8_half[:, o0:o0 + NFREE], psums[i][j],
                                1.0 / S2_SCALE)
                        else:
                            last_copy["act"] = nc.scalar.mul(
                                eo8_half[:, o0:o0 + NFREE], psums[i][j],
                                1.0 / S2_SCALE)
                # stage ag_in on the fast HWDGE rings (the SWDGE ring is
                # ~25 GB/s: ~20 us per 0.5 MB half, which pushed each AG's
                # start late enough that a slow AG1 draw stalled stage-2
                # half 1 by ~12 us).  ch0's staging is deferred into ch1's
                # block-1 slot so its ring slots sit BEHIND the
                # boundary-critical ch1-blk0 loads; the collective carries
                # explicit deps since it no longer shares the SWDGE queue
                # with its staging.
                def emit_ag(ch=ch, eo8_half=eo8_half):
                    dmas = [
                        nc.scalar.dma_start(
                            out=ag_in[ch][:, :2 * D_OUT],
                            in_=eo8_half[:, :2 * D_OUT]),
                        nc.sync.dma_start(
                            out=ag_in[ch][:, 2 * D_OUT:],
                            in_=eo8_half[:, 2 * D_OUT:]),
                    ]
                    cc = nc.gpsimd.collective_compute(
                        "AllGather", mybir.AluOpType.bypass,
                        replica_groups=[list(range(N_CORES))],
                        ins=[ag_in[ch][:].opt()], outs=[ag_out[ch][:].opt()])
                    for dm in dmas:
                        add_dep_helper(cc.ins, dm.ins, False, "AG after staging")
                if ch == 0:
                    pending_ag[0] = emit_ag
                else:
                    emit_ag()

            # ------------- Stages 2+3, interleaved ---------------------------
            # Emission order: s3 blocks 0-5 (covers AG0 tail), local-bf16 twin
            # of half 0, s2 fp8 blocks 0-7, s3 blocks 6-7 (covers AG1 tail),
            # local-bf16 twin of half 1, s2 fp8 blocks 8-15 (block 15
            # finishes: PSUM->SBUF copies + output DMA).
            psums = psum_tiles("s23")
            s3_dr = s3_dt == dt.float8e4
            out_sb = p_out.tile([128, 4, D_OUT], dt.float32)
            eoag_state = {"prev": None}
            S3_SPLIT = 6

            def s2_local_block(h):
                # this core's own eo chunk, bf16, no collective dependency
                t = p_cwl.tile([128, SUB * S_LOC], io_dt, tag="cwl", name=f"cw16_{h}")
                nc.sync.dma_start(
                    out=t, in_=cwT16[:, h * SUB * S_LOC:(h + 1) * SUB * S_LOC])
                cw_t = unflat(t, SUB)
                for sub in range(SUB):
                    for i in range(4):
                        mm_pair(psums[i],
                                cw_t[:, sub, i * 128:(i + 1) * 128],
                                lambda j, sub=sub: eo16[h][:, sub, j * NFREE:(j + 1) * NFREE],
                                start=False, stop=False)

            def load_eoag(blk):
                half = blk // 8              # ag chunk this block reads
                rk = blk % 8                 # rank whose eo chunk this is
                if blk not in cw_tiles:
                    load_cw(blk)
                eo_f = p_eoag.tile([128, SUB * D_OUT], dt.float8e4, tag="eoag",
                                   name=f"eoag_{blk}")
                # alternate rings: stage-1's ft/we streams are done by now, so
                # the sync ring is free to carry half the eoag supply
                q = nc.scalar if blk % 2 else nc.sync
                eoag_dma = q.dma_start(
                    out=eo_f, in_=ag_out[half][rk * 128:(rk + 1) * 128, :])
                # pin each queue's first eoag after stage-1's LAST PSUM->SBUF
                # copies and the operand loads, then keep block order within
                # the queue.  Without this the scheduler hoists an AG-gated
                # dispatch ahead of the half-1 copies: the engine FIFO
                # head-of-line blocks on the collective, PSUM turnaround
                # stalls, and every core's stage-1 tail (so the collective
                # itself) serializes (measured 68us PE stall).
                key = "prev_s" if blk % 2 else "prev_y"
                prev = eoag_state.get(key)
                if prev is not None:
                    add_dep_helper(eoag_dma.ins, prev.ins, False,
                                   "eoag in block order per queue")
                else:
                    for root in (last_copy["dve"], last_copy["act"],
                                 eoag_state.get("root") or last_we[0]):
                        if root is not None:
                            add_dep_helper(eoag_dma.ins, root.ins, False,
                                           "first eoag after stage-1 tail")
                eoag_state[key] = eoag_dma
                return unflat(eo_f, SUB)

            def s2_block(blk):
                eo_t = load_eoag(blk)
                cw_t = cw_tiles[blk]
                for s2i in range(SUB // 2):
                    for i in range(4):
                        mm_pair(psums[i],
                                cw_t[:, 2 * s2i:2 * s2i + 2, i * 128:(i + 1) * 128],
                                lambda j, s2i=s2i: eo_t[:, 2 * s2i:2 * s2i + 2,
                                                        j * NFREE:(j + 1) * NFREE],
                                start=False, stop=False,
                                perf_mode=mybir.MatmulPerfMode.DoubleRow)
                return cw_t, eo_t

            def s2_tail(blks):
                # bank-major sweep over the last blocks: each (i, j0/j1) bank
                # pair finishes ~4.2 us apart, so its PSUM->SBUF copies (split
                # DVE/ACT) and output DMAs (split sync/scalar) hide behind the
                # remaining matmuls instead of stacking up after the last one
                tiles = []
                for b in blks:
                    eo_t = load_eoag(b)
                    tiles.append((cw_tiles[b], eo_t))
                for i in range(4):
                    for bi, (cw_t, eo_t) in enumerate(tiles):
                        for s2i in range(SUB // 2):
                            mm_pair(psums[i],
                                    cw_t[:, 2 * s2i:2 * s2i + 2, i * 128:(i + 1) * 128],
                                    lambda j, s2i=s2i, eo_t=eo_t: eo_t[:, 2 * s2i:2 * s2i + 2,
                                                                      j * NFREE:(j + 1) * NFREE],
                                    start=False,
                                    stop=(bi == len(tiles) - 1 and s2i == SUB // 2 - 1),
                                    perf_mode=mybir.MatmulPerfMode.DoubleRow)
                    for j in range(NJ):
                        if SPLIT_COPY and j % 2:
                            nc.scalar.copy(out_sb[:, i, j * NFREE:(j + 1) * NFREE],
                                           psums[i][j])
                        else:
                            nc.vector.tensor_copy(
                                out=out_sb[:, i, j * NFREE:(j + 1) * NFREE],
                                in_=psums[i][j])
                        q = nc.scalar if (SPLIT_COPY and j % 2) else nc.sync
                        q.dma_start(
                            out=out[i * 128:(i + 1) * 128, j * NFREE:(j + 1) * NFREE]
                                .rearrange("(n p) d -> p n d", p=128),
                            in_=out_sb[:, i:i + 1, j * NFREE:(j + 1) * NFREE])

            def s3_blocks(b0, b1, start):
                if b0 == 0:
                    # issue the remaining stage-3 operand loads now, BEFORE any
                    # AG-gated eoag dispatch enters the rings (ring descriptors
                    # process in order; anything queued behind a gated eoag
                    # waits for the collective)
                    for blk in range(NBLK):
                        if blk not in ri_tiles:
                            load_ri(blk)
                        if blk not in rw_tiles:
                            eoag_state["root"] = load_rw(blk)
                for blk in range(b0, b1):
                    if blk not in ri_tiles:
                        load_ri(blk)
                    ri_t = ri_tiles[blk]
                    if blk not in rw_tiles:
                        eoag_state["root"] = load_rw(blk)
                    rw_t = rw_tiles[blk]
                    if s3_dr:
                        for s2 in range(SUB // 2):
                            for i in range(4):
                                mm_pair(psums[i],
                                        ri_t[:, 2 * s2:2 * s2 + 2, i * 128:(i + 1) * 128],
                                        lambda j, s2=s2: rw_t[:, 2 * s2:2 * s2 + 2,
                                                              j * NFREE:(j + 1) * NFREE],
                                        start=(start and blk == b0 and s2 == 0),
                                        stop=False,
                                        perf_mode=mybir.MatmulPerfMode.DoubleRow)
                    else:
                        for sub in range(SUB):
                            for i in range(4):
                                mm_pair(psums[i],
                                        ri_t[:, sub, i * 128:(i + 1) * 128],
                                        lambda j, sub=sub: rw_t[:, sub, j * NFREE:(j + 1) * NFREE],
                                        start=(start and blk == b0 and sub == 0),
                                        stop=False)

            s3_blocks(0, S3_SPLIT, start=True)       # covers AG0's tail
            if LOCAL_BF16:
                s2_local_block(0)                    # AG-independent filler
            for blk in range(8):                     # stage-2 fp8 half 0
                s2_block(blk)
            s3_blocks(S3_SPLIT, NBLK, start=False)   # covers AG1's tail
            if LOCAL_BF16 == 2:
                s2_local_block(1)                    # AG-independent filler
            for blk in range(8, 10):                 # stage-2 fp8 half 1
                s2_block(blk)
            s2_tail(list(range(10, ECT // SUB)))     # bank-major finish

    nc.finalize()
    if PAIR_SKIP:
        d, n_del = _pairskip_ldweights_json(json.loads(nc.to_json_bytes()))
        fused = json.dumps(d).encode()
        nc.to_json_bytes = lambda: fused
    return nc


def _get_prog(ldw_opt):
    key = (ldw_opt,)
    if key not in _prog_cache:
        if ldw_opt:
            _patch_ldw_opt()
        _prog_cache[key] = _build(ldw_opt)
    return _prog_cache[key]


def _swz(a, nblk, nsub):
    """[nblk*nsub*128, d] contraction-major -> partition-major [128, nblk*nsub*d]
    with each partition's data contiguous (fat DMA descriptors)."""
    d = a.shape[1]
    return np.ascontiguousarray(
        a.reshape(nblk, nsub, 128, d).transpose(2, 0, 1, 3).reshape(128, nblk * nsub * d))


def _prep_in_maps(inputs, expert_w, residual_w, combine_weights, residual_weight):
    np_dt = BF16
    front = inputs[:E * C].reshape(E, C, D_IN)
    resid = inputs[E * C:]                       # [TOK, D_IN]
    rwt = residual_weight.reshape(TOK, 2)
    w0, w1 = rwt[:, 0], rwt[:, 1]

    s3_np = F8 if FP8_S3 else np_dt
    rw_scale = FP8_S if FP8_S3 else 1.0
    rw_sw = _swz((residual_w * rw_scale).astype(s3_np), 8, 4)            # [128, 8*4*1024]
    resid_s = resid * (w1[:, None] / rw_scale)   # fold w1 and 1/S (fp32)
    in_maps = []
    for r in range(N_CORES):
        sl = slice(r * S_LOC, (r + 1) * S_LOC)
        fT = front[r].T.astype(np_dt)                                    # [D_IN, C]
        # fT layout [128, (ch, blk, sub, c)]: the kernel reads c-halves
        fT_sw = np.ascontiguousarray(
            fT.reshape(16, 2, 128, 2, CH).transpose(2, 3, 0, 1, 4).reshape(128, -1))
        we_sw = _swz(expert_w[r].astype(np_dt), 16, 2)                   # [128, 16*2*1024]
        # centered combine weights: v = (cw - 0.5) * w0; the removed mean is
        # added back exactly on the host (rank-1 term, see _run)
        cw_c = (combine_weights[sl] - 0.5) * w0[sl, None, None]          # [S_LOC, E, C]
        # contraction rows ordered (c-half chunk, expert, c-within-half) to
        # match the chunked AllGather's concatenation
        cwT_f32 = (cw_c.reshape(S_LOC, E, 2, CH).transpose(2, 1, 3, 0)
                   .reshape(E * C, S_LOC))
        cw8_src = cwT_f32 * S2_SCALE
        if LOCAL_BF16:
            # this core's own chunks run through the local bf16 path; zero
            # their fp8 slots (the SPMD program still executes them)
            loc_blocks = [r, 8 + r][:LOCAL_BF16]
            cw8_src = cw8_src.copy()
            for b in loc_blocks:
                cw8_src[b * 512:(b + 1) * 512] = 0.0
            cw16 = np.concatenate(
                [cwT_f32[b * 512:(b + 1) * 512]
                 for b in sorted(loc_blocks)], axis=0)
            cwT16_sw = _swz(cw16.astype(np_dt), LOCAL_BF16, 4)
        cwT8_sw = _swz(cw8_src.astype(F8), 16, 4)                        # [128, 16*4*512]
        riT_sw = _swz(resid_s[sl].T.astype(s3_np), 8, 4)                 # [128, 8*4*512]
        m = {"fT": fT_sw, "we": we_sw, "cwT8": cwT8_sw, "riT": riT_sw, "rw": rw_sw}
        if LOCAL_BF16:
            m["cwT16"] = cwT16_sw
        in_maps.append(m)
    return in_maps


def _run(inputs, expert_w, expert_b, residual_w, residual_b,
         combine_weights, residual_weight, ldw_opt=None, trace=False, mode=None):
    import jax
    try:
        if jax.config.jax_compilation_cache_dir is None:
            jax.config.update("jax_compilation_cache_dir", "/tmp/jax_cache_trn_moe")
            jax.config.update("jax_persistent_cache_min_compile_time_secs", 0.5)
    except Exception:
        pass
    from concourse.bass_utils import run_bass_kernel_spmd

    ldw_opt = LDW_OPT if ldw_opt is None else ldw_opt
    inputs = np.asarray(inputs, dtype=np.float32)
    expert_w = np.asarray(expert_w, dtype=np.float32)
    expert_b = np.asarray(expert_b, dtype=np.float32)
    residual_w = np.asarray(residual_w, dtype=np.float32)
    residual_b = np.asarray(residual_b, dtype=np.float32)
    combine_weights = np.asarray(combine_weights, dtype=np.float32)
    residual_weight = np.asarray(residual_weight, dtype=np.float32)

    nc = _get_prog(ldw_opt)
    in_maps = _prep_in_maps(inputs, expert_w, residual_w, combine_weights,
                            residual_weight)
    res = run_bass_kernel_spmd(nc, in_maps, list(range(N_CORES)), trace=trace)
    out = np.concatenate([res.results[r]["out"] for r in range(N_CORES)], axis=0)

    rwt = residual_weight.reshape(TOK, 2)
    # exact centering term: 0.5 * w0[s] * colsum(front @ W)[m], float64
    front = inputs[:E * C].reshape(E, C, D_IN)
    G = np.zeros(D_OUT, np.float64)
    for e in range(E):
        G += front[e].sum(axis=0).astype(np.float64) @ expert_w[e].astype(np.float64)
    out = out + (0.5 * rwt[:, 0:1]) * G[None, :].astype(np.float32)
    # exact bias contributions (zero in practice, but keep the math honest)
    if residual_b.any():
        out = out + rwt[:, 1:2] * residual_b[None, :]
    if expert_b.any():
        cs = combine_weights.sum(axis=2)                    # [TOK, E]
        out = out + rwt[:, 0:1] * (cs @ expert_b)
    return out.reshape(B, S, D_OUT).astype(np.float32), res


def kernel(**kw):
    out, _ = _run(**kw)
    return out


# revision 42
# speedup vs baseline: 1.0542x; 1.0135x over previous
"""Expert-parallel MoE "behind" block + residual on 8 Trainium2 NeuronCores.

Reference computation (fp32):
    front      = inputs[:E*C].reshape(E, C, D_IN)
    expert_out = einsum("ecd,edm->ecm", front, expert_w) + expert_b
    combined   = einsum("sec,ecm->sm", combine_weights, expert_out)
    resid      = inputs[E*C:] @ residual_w + residual_b
    out        = combined * w0[:, None] + resid * w1[:, None]

Sharding (8 cores):
  Stage 1 (expert-parallel, bf16): core e computes eo_e = front_e @ W_e
  [C, D_OUT] in two c-halves; each half is copied out of PSUM twice — once
  to bf16 (kept locally) and once to fp8 e4m3 with a 1/4 scale — and the
  fp8 copy is AllGathered (halved collective payload vs bf16: ~38 us/chunk).
  Stage 3 (token-parallel residual): fp8 e4m3 DoubleRow as before.
  Stage 2 (token-parallel combine): ALL 16 k-blocks run fp8 e4m3 DoubleRow
  (2 k-rows/cycle) against the AllGathered fp8 eo.  Two accuracy devices
  make this fit the 2e-2 rel-l2 bar (measured 1.664e-2 baseline budget):

  * Mean-centering: cw is uniform[0,1); the device matmuls use
    v = (cw - 0.5)*w0 (rms halved), which halves BOTH fp8 error terms
    (q(v)·eo and v·q(eo)); the removed mean is an exact host-side rank-1
    term  0.5*w0[s] * colsum(eo)[m]  with colsum(eo) = sum_e
    (colsum front_e) @ W_e computed on host in float64 (67 MFLOP, same
    spirit as the existing exact expert_b/residual_b foldbacks).
  * Local-bf16 twins: each core's OWN two eo chunks (half h, rank r) never
    need the collective — they are read from the local bf16 eo_half and
    contracted in bf16.  SPMD runs one program on all cores, so the fp8
    slots for those two blocks still execute but with their cw data zeroed
    host-side (exact zero contribution), and two extra bf16 blocks carry
    the real values.  Costs 2x4.2 us of zero-matmuls; removes the two
    largest error terms per core.

  Default config runs ALL 16 blocks fp8 (LOCAL_BF16=0): measured rel-l2
  1.8876e-2 on HW, bit-identical to the numpy emulation and deterministic
  across runs (fixed seed-0 inputs; the metric averages 4M outputs, so it
  is also insensitive to input resampling).  LOCAL_BF16=2 gives 1.767e-2
  at +25 us.

All DRAM operands are host-swizzled partition-major as before (fat DMA
descriptors).  Perf notes (measured on these trn2 cores):
  * Every 512-col MM cadences at exactly 263 ns (512 cycles at the
    sustained 13/16-throttled 1.95 GHz clock), bf16 and fp8-DR alike; the
    kernel is cadence-bound: 894 MMs = 235 us + ~12 us boot head + ~2 us
    gaps + ~2 us tail = 252-262 us (vs 316-326 us baseline).  Some runs
    draw the full 2.4 GHz clock (no SW throttle) — the ring balancing
    below is what lets those run supply-clean.
  * Stage-1 ft/we tile loads alternate between the two HWDGE rings per
    block: the we stream alone is ~8.6 us/MB-tile x 16 = 137 us, which
    out-runs stage-1's PE time on one ring.  S1SUB=4 (4 k-subtiles per
    DMA) halves descriptor counts; ri/rw/cw ship as 2-block granules.
  * eoag loads alternate sync/scalar, with each queue's FIRST eoag pinned
    after stage-1's last PSUM copies — an AG-gated dispatch hoisted ahead
    of them head-of-line blocks the engine FIFO, stalls PSUM turnaround,
    and serializes the collective across all cores (measured 68 us).
  * The two fp8 AllGathers run serially on the CC engine (~100 us
    combined); stage-3 splits 6+2 around stage-2 half 0 to cover their
    tails.
  * The last 6 combine blocks run bank-major so each PSUM bank's output
    copy + DMA hides behind the remaining matmuls.
  * 16 warmup matmuls release the PE HAM clock gate during the ~12 us
    boot+DMA head (N_WARM=0 measured worse: mid-stage-1 HAM wobble).
  * _pairskip drops redundant Ldweights.

Env knobs: TRN_LOCAL_BF16=0 (default; 2 = bf16 twins, lower error),
TRN_FP8_S3=1, TRN_WARM=16, TRN_SKIP_LDW=1, TRN_PAIR_SKIP=1.
"""

import json
import os
import numpy as np
import ml_dtypes

E, C, D_IN, D_OUT = 8, 1024, 4096, 1024
B, S = 2, 2048
TOK = B * S                 # 4096 tokens
N_CORES = 8
S_LOC = TOK // N_CORES      # 512 tokens per core
CH = C // 2                 # c-half = 512
BF16 = ml_dtypes.bfloat16
F8 = ml_dtypes.float8_e4m3

LDW_OPT = os.environ.get("TRN_LDW_OPT", "0") == "1"
SKIP_LDW = os.environ.get("TRN_SKIP_LDW", "1") == "1"
PAIR_SKIP = os.environ.get("TRN_PAIR_SKIP", "1") == "1"
FP8_S3 = os.environ.get("TRN_FP8_S3", "1") == "1"
N_WARM = int(os.environ.get("TRN_WARM", "28"))
SPLIT_COPY = os.environ.get("TRN_SPLIT_COPY", "1") == "1"
# 2 = each core contracts its own two eo chunks in bf16 (fp8 slots zeroed);
# 0 = all 16 combine blocks in fp8 (rel-l2 1.888e-2 vs 1.767e-2, both
# deterministic on the fixed seed-0 inputs; 0 saves ~25 us of PE time)
LOCAL_BF16 = int(os.environ.get("TRN_LOCAL_BF16", "0"))
# fp8 e4m3 operand scalings (products exact):
FP8_S = 4.0                 # stage 3: riT*(1/S), rw*S
S2_SCALE = 4.0              # stage 2: cw8*S, eo8*(1/S)

_prog_cache = {}


def _pairskip_ldweights_json(d):
    """Delete Ldweights that reload the weights already in the PE array.

    bass unconditionally emits Ldweights+Matmult for every matmul, so the
    mm_pair weight-reuse never took effect.  Measured on HW: a 512-row bf16
    MM whose weights must load first has a 263 ns cadence vs 216 ns when the
    array already holds them.  Emulate the array state over the PE queue and
    drop any Ldweights whose weights AP exactly matches the one currently
    loaded; its semaphore wait (rare) moves to the next PE instruction or an
    EventSemaphore.
    """
    n_del = 0
    for fn in d["functions"]:
        for blk in fn["blocks"]:
            out, loaded, carry_wait = [], None, []
            for ins in blk["instructions"]:
                op = ins["opcode"]
                if op == "Ldweights":
                    sig = json.dumps(ins["ins"][0], sort_keys=True)
                    if sig == loaded:
                        w = (ins.get("sync_info") or {}).get("on_wait") or []
                        carry_wait.extend(w)
                        n_del += 1
                        continue
                    loaded = sig
                elif op == "Matmult" and carry_wait:
                    si = ins.get("sync_info") or {"on_update": [], "on_wait": []}
                    ins["sync_info"] = si
                    if not (si.get("on_wait") or []):
                        si["on_wait"] = [carry_wait.pop(0)]
                    for w in carry_wait:
                        out.append({
                            "debug": ins.get("debug", 0), "engine": "PE",
                            "ins": [], "outs": [],
                            "name": ins["name"] + f"_cw{len(out)}",
                            "opcode": "EventSemaphore",
                            "sync_info": {"on_update": [], "on_wait": [w]},
                        })
                    carry_wait = []
                out.append(ins)
            assert not carry_wait
            blk["instructions"] = out
    return d, n_del


def _patch_ldw_opt():
    from concourse import bass_utils
    if getattr(bass_utils, "_ldw_opt_patched", False):
        return
    orig = bass_utils.run_command

    def patched(argv, **kw):
        argv = ["--enable-ldw-opt=true" if a == "--enable-ldw-opt=false" else a
                for a in argv]
        return orig(argv, **kw)

    bass_utils.run_command = patched
    bass_utils._ldw_opt_patched = True


def _build(ldw_opt):
    import concourse.bass as bass  # noqa: F401
    import concourse.mybir as mybir
    from concourse import bacc
    from concourse.tile import TileContext, add_dep_helper

    dt = mybir.dt
    io_dt = dt.bfloat16

    nc = bacc.Bacc("TRN2", target_bir_lowering=False, debug=False, num_devices=N_CORES)

    s3_dt = dt.float8e4 if FP8_S3 else io_dt
    fT = nc.declare_dram_parameter("fT", [128, D_IN // 128 * C], io_dt, isOutput=False)
    we = nc.declare_dram_parameter("we", [128, D_IN // 128 * D_OUT], io_dt, isOutput=False)
    cwT8 = nc.declare_dram_parameter("cwT8", [128, E * C // 128 * S_LOC], dt.float8e4,
                                     isOutput=False)
    cwT16 = (nc.declare_dram_parameter("cwT16", [128, LOCAL_BF16 * 4 * S_LOC], io_dt,
                                       isOutput=False) if LOCAL_BF16 else None)
    riT = nc.declare_dram_parameter("riT", [128, D_IN // 128 * S_LOC], s3_dt, isOutput=False)
    rw = nc.declare_dram_parameter("rw", [128, D_IN // 128 * D_OUT], s3_dt, isOutput=False)
    out = nc.declare_dram_parameter("out", [S_LOC, D_OUT], dt.float32, isOutput=True)

    # variant tag so differently-compiled builds never share a jax cache entry
    nc.dram_tensor(
        f"variant_v2_{int(ldw_opt)}_{int(SKIP_LDW)}_{int(PAIR_SKIP)}"
        f"_{int(FP8_S3)}_{N_WARM}_{int(SPLIT_COPY)}_loc{LOCAL_BF16}",
        [1, 1], dt.float32)

    ag_in = [nc.dram_tensor(f"ag_in{h}", [128, 4 * D_OUT], dt.float8e4) for h in range(2)]
    ag_out = [nc.dram_tensor(f"ag_out{h}", [N_CORES * 128, 4 * D_OUT], dt.float8e4,
                             addr_space="Shared") for h in range(2)]

    KT = D_IN // 128            # 32 contraction tiles
    SUB = 4                     # k-subtiles per DMA'd block
    NBLK = KT // SUB            # 8 stage-3 blocks
    ECT = (E * C) // 128        # 64 combine contraction tiles
    NFREE = 512                 # PSUM bank cap: 512 fp32 out elements = 2 KiB
    NJ = D_OUT // NFREE

    S1SUB = 4                   # k-subtiles per stage-1 DMA'd block: 4KB/8KB
    S1BLK = KT // S1SUB         # descriptors halve ring time vs 2-subtile
    B_FT, B_WE, B_RI, B_RW, B_CW, B_EOAG = 7, 6, 4, 4, 3, 8
    NPRE = 1                    # stage-3/2 block-pairs prefetched during stage 1
    with TileContext(nc) as tc:
        with tc.tile_pool(name="p_ft", bufs=B_FT) as p_ft, \
             tc.tile_pool(name="p_we", bufs=B_WE) as p_we, \
             tc.tile_pool(name="p_ri", bufs=B_RI) as p_ri, \
             tc.tile_pool(name="p_rw", bufs=B_RW) as p_rw, \
             tc.tile_pool(name="p_cw", bufs=B_CW) as p_cw, \
             tc.tile_pool(name="p_cwl", bufs=2) as p_cwl, \
             tc.tile_pool(name="p_eoag", bufs=B_EOAG) as p_eoag, \
             tc.tile_pool(name="p_eo", bufs=2) as p_eo, \
             tc.tile_pool(name="p_eo8", bufs=2) as p_eo8, \
             tc.tile_pool(name="p_out", bufs=1) as p_out, \
             tc.tile_pool(name="p_warm", bufs=1) as p_warm, \
             tc.tile_pool(name="psum", bufs=1, space="PSUM") as p_ps:

            def mm_pair(psrow, lhsT_ap, rhs_of_j, start, stop, perf_mode=None):
                """Two matmuls sharing one stationary operand: the second
                skips its LDWEIGHTS and is order-pinned after the first."""
                prev = None
                for j in range(NJ):
                    m = nc.tensor.matmul(psrow[j], lhsT_ap, rhs_of_j(j),
                                         start=start, stop=stop,
                                         perf_mode=perf_mode)
                    if j > 0 and SKIP_LDW:
                        m.ins.ldweights = False
                        add_dep_helper(m.ins, prev.ins, False, "weight-reuse pair order")
                    prev = m

            def psum_tiles(tagp):
                return [[p_ps.tile([128, NFREE], dt.float32,
                                   name=f"{tagp}_{i}_{j}", tag=f"ps_{i}_{j}")
                         for j in range(NJ)] for i in range(4)]

            # ------------- Warmup: release the PE HAM clock gate -------------
            if N_WARM:
                warm_t = p_warm.tile([128, 128 + NFREE], io_dt)
                nc.vector.memset(warm_t, 0)
                warm_ps = p_ps.tile([128, NFREE], dt.float32,
                                    name="warm_ps", tag="ps_0_0")
                for _ in range(N_WARM):
                    nc.tensor.matmul(warm_ps, warm_t[:, :128],
                                     warm_t[:, 128:128 + NFREE],
                                     start=True, stop=True)

            ri_tiles, rw_tiles, cw_tiles = {}, {}, {}
            unflat = lambda t, n: t.rearrange("p (n d) -> p n d", n=n)

            # 2-block granule loads: each partition's data for consecutive
            # blocks is contiguous in the swizzled DRAM layouts, so pairing
            # blocks halves the descriptor count on the rings
            def load_ri(blk):
                b0 = blk - blk % 2
                t = p_ri.tile([128, 2 * SUB * S_LOC], s3_dt, tag="ri", name=f"ri_{b0}")
                nc.sync.dma_start(
                    out=t, in_=riT[:, b0 * SUB * S_LOC:(b0 + 2) * SUB * S_LOC])
                v = unflat(t, 2 * SUB)
                ri_tiles[b0] = v[:, :SUB]
                ri_tiles[b0 + 1] = v[:, SUB:]

            def load_rw(blk, q=None):
                b0 = blk - blk % 2
                t = p_rw.tile([128, 2 * SUB * D_OUT], s3_dt, tag="rw", name=f"rw_{b0}")
                d = (q or nc.scalar).dma_start(
                    out=t, in_=rw[:, b0 * SUB * D_OUT:(b0 + 2) * SUB * D_OUT])
                v = unflat(t, 2 * SUB)
                rw_tiles[b0] = v[:, :SUB]
                rw_tiles[b0 + 1] = v[:, SUB:]
                return d

            def load_cw(blk):
                b0 = blk - blk % 2
                t = p_cw.tile([128, 2 * SUB * S_LOC], dt.float8e4, tag="cw",
                              name=f"cw8_{b0}")
                nc.sync.dma_start(
                    out=t, in_=cwT8[:, b0 * SUB * S_LOC:(b0 + 2) * SUB * S_LOC])
                v = unflat(t, 2 * SUB)
                cw_tiles[b0] = v[:, :SUB]
                cw_tiles[b0 + 1] = v[:, SUB:]

            # ------------- Stage 1: eo_e = fT.T @ we, by c-halves ------------
            last_we = [None]
            last_copy = {"dve": None, "act": None}
            eo16 = {}
            s1_tiles = {}

            def load_s1(ch, blk):
                if (ch, blk) in s1_tiles:
                    return s1_tiles[(ch, blk)]
                f0 = (ch * S1BLK + blk) * (S1SUB * CH)
                w0_ = blk * (S1SUB * D_OUT)
                ft_f = p_ft.tile([128, S1SUB * CH], io_dt, tag="ft", name=f"ft_{ch}_{blk}")
                we_f = p_we.tile([128, S1SUB * D_OUT], io_dt, tag="we", name=f"we_{ch}_{blk}")
                if ch == 0 and blk == 0:
                    # the first block gates the whole pipeline: balance its
                    # 1.5 MB across the two fast HWDGE rings (~6.5 us each
                    # at ~116 GB/s).  Keep it OFF the gpsimd SWDGE ring
                    # (~25 GB/s — routing a 256 KB ft half there measured
                    # first-matmul at ~23 us instead of ~13).
                    nc.sync.dma_start(out=ft_f, in_=fT[:, f0:f0 + S1SUB * CH])
                    last_we[0] = nc.scalar.dma_start(
                        out=we_f[0:96, :],
                        in_=we[0:96, w0_:w0_ + S1SUB * D_OUT])
                    nc.sync.dma_start(
                        out=we_f[96:128, :],
                        in_=we[96:128, w0_:w0_ + S1SUB * D_OUT])
                else:
                    # balance the two HWDGE rings: the we stream alone is
                    # ~8.6us/tile (1 MB at ~116 GB/s) x 16 = 137us, which
                    # binds stage 1 on one ring; alternating ft/we between
                    # the rings puts ~103us on each
                    qa, qb = ((nc.sync, nc.scalar) if blk % 2 == ch
                              else (nc.scalar, nc.sync))
                    qa.dma_start(out=ft_f, in_=fT[:, f0:f0 + S1SUB * CH])
                    last_we[0] = qb.dma_start(
                        out=we_f, in_=we[:, w0_:w0_ + S1SUB * D_OUT])
                s1_tiles[(ch, blk)] = (unflat(ft_f, S1SUB), unflat(we_f, S1SUB))
                return s1_tiles[(ch, blk)]

            for ch in range(2):
                psums = psum_tiles(f"s1h{ch}")
                for blk in range(S1BLK):
                    # pre-issue the FIRST stage-3/2 operand pairs late in
                    # half 0, one per block (early half-0 slots would steal
                    # ring time from the ft/we ramp; half-1 slots dilute the
                    # boundary supply; later pairs arrive in time via the
                    # stage-2/3 emission path)
                    if ch == 0 and S1BLK - 1 - 3 * NPRE <= blk < S1BLK - 1:
                        k, r = divmod(S1BLK - 1 - blk - 1, 3)
                        if k < NPRE:
                            # 2-block granules: k-th pre-issue covers blocks
                            # 2k, 2k+1 of each operand
                            (load_ri,
                             lambda b: load_rw(b, q=nc.scalar),
                             load_cw)[r](2 * k)
                    # (hoisting ch1-blk0/1 loads into half 0 measured WORSE:
                    # the rings run at ~1.3x of stage-1's consumption, so a
                    # 3 MB insertion starves half-0's own tail for ~8 us)
                    ft_t, we_t = load_s1(ch, blk)
                    if blk < S1BLK - 1:
                        for sub in range(S1SUB):
                            kt = blk * S1SUB + sub
                            for i in range(4):
                                mm_pair(psums[i],
                                        ft_t[:, sub, i * 128:(i + 1) * 128],
                                        lambda j, sub=sub: we_t[:, sub, j * NFREE:(j + 1) * NFREE],
                                        start=(kt == 0), stop=False)
                eo8_half = p_eo8.tile([128, 4 * D_OUT], dt.float8e4, tag="eo8",
                                      name=f"eo8_{ch}")
                if LOCAL_BF16:
                    eo_half = p_eo.tile([128, 4 * D_OUT], io_dt, tag="eo",
                                        name=f"eo_{ch}")
                    eo16[ch] = unflat(eo_half, 4)
                # last block: finish each PSUM bank in turn; copy to bf16
                # (DVE, kept local) and scaled fp8 (ACT, AllGathered)
                for i in range(4):
                    for j in range(NJ):
                        for sub in range(S1SUB):
                            nc.tensor.matmul(
                                psums[i][j],
                                ft_t[:, sub, i * 128:(i + 1) * 128],
                                we_t[:, sub, j * NFREE:(j + 1) * NFREE],
                                start=False, stop=(sub == S1SUB - 1))
                        o0 = i * D_OUT + j * NFREE
                        if LOCAL_BF16:
                            nc.vector.tensor_copy(
                                out=eo_half[:, o0:o0 + NFREE], in_=psums[i][j])
                        # split the fp8 scaled copies across DVE and ACT so the
                        # 8-bank turnaround doesn't serialize on one engine
                        # (single-engine measured a 4.4us PE stall + HAM
                        # re-throttle at the half boundary)
                        if SPLIT_COPY and j % 2 == 0 and not LOCAL_BF16:
                            last_copy["dve"] = nc.vector.tensor_scalar_mul(
                                eo8_half[:, o0:o0 + NFREE], psums[i][j],
                                1.0 / S2_SCALE)
                        else:
                            last_copy["act"] = nc.scalar.mul(
                                eo8_half[:, o0:o0 + NFREE], psums[i][j],
                                1.0 / S2_SCALE)
                # stage ag_in on the fast HWDGE rings (the SWDGE ring is
                # ~25 GB/s: ~20 us per 0.5 MB half, which pushed each AG's
                # start late enough that a slow AG1 draw stalled stage-2
                # half 1 by ~12 us).  ch0's staging is deferred into ch1's
                # block-1 slot so its ring slots sit BEHIND the
                # boundary-critical ch1-blk0 loads; the collective carries
                # explicit deps since it no longer shares the SWDGE queue
                # with its staging.
                if ch == 0:
                    # half 0: SWDGE staging (~20 us) — the HWDGE rings have no
                    # slack mid-stage-1 (inserting 0.5 MB there starves the
                    # ft/we stream), and AG0 has ~50 us of slack anyway
                    dmas = [nc.gpsimd.dma_start(
                        out=ag_in[ch][:, k * 2 * D_OUT:(k + 1) * 2 * D_OUT],
                        in_=eo8_half[:, k * 2 * D_OUT:(k + 1) * 2 * D_OUT])
                        for k in range(2)]
                else:
                    # half 1: the HWDGE rings are idle once stage 1 ends, and
                    # AG1's completion is the tight dependency (a slow AG1
                    # draw stalled stage-2 half 1 by ~12 us when this staging
                    # sat on the ~25 GB/s SWDGE ring)
                    dmas = [
                        nc.scalar.dma_start(
                            out=ag_in[ch][:, :2 * D_OUT],
                            in_=eo8_half[:, :2 * D_OUT]),
                        nc.sync.dma_start(
                            out=ag_in[ch][:, 2 * D_OUT:],
                            in_=eo8_half[:, 2 * D_OUT:]),
                    ]
                cc = nc.gpsimd.collective_compute(
                    "AllGather", mybir.AluOpType.bypass,
                    replica_groups=[list(range(N_CORES))],
                    ins=[ag_in[ch][:].opt()], outs=[ag_out[ch][:].opt()])
                for dm in dmas:
                    add_dep_helper(cc.ins, dm.ins, False, "AG after staging")

            # ------------- Stages 2+3, interleaved ---------------------------
            # Emission order: s3 blocks 0-5 (covers AG0 tail), local-bf16 twin
            # of half 0, s2 fp8 blocks 0-7, s3 blocks 6-7 (covers AG1 tail),
            # local-bf16 twin of half 1, s2 fp8 blocks 8-15 (block 15
            # finishes: PSUM->SBUF copies + output DMA).
            psums = psum_tiles("s23")
            s3_dr = s3_dt == dt.float8e4
            out_sb = p_out.tile([128, 4, D_OUT], dt.float32)
            eoag_state = {"prev": None}
            S3_SPLIT = 6

            def s2_local_block(h):
                # this core's own eo chunk, bf16, no collective dependency
                t = p_cwl.tile([128, SUB * S_LOC], io_dt, tag="cwl", name=f"cw16_{h}")
                nc.sync.dma_start(
                    out=t, in_=cwT16[:, h * SUB * S_LOC:(h + 1) * SUB * S_LOC])
                cw_t = unflat(t, SUB)
                for sub in range(SUB):
                    for i in range(4):
                        mm_pair(psums[i],
                                cw_t[:, sub, i * 128:(i + 1) * 128],
                                lambda j, sub=sub: eo16[h][:, sub, j * NFREE:(j + 1) * NFREE],
                                start=False, stop=False)

            def load_eoag(blk):
                half = blk // 8              # ag chunk this block reads
                rk = blk % 8                 # rank whose eo chunk this is
                if blk not in cw_tiles:
                    load_cw(blk)
                eo_f = p_eoag.tile([128, SUB * D_OUT], dt.float8e4, tag="eoag",
                                   name=f"eoag_{blk}")
                # alternate rings: stage-1's ft/we streams are done by now, so
                # the sync ring is free to carry half the eoag supply
                q = nc.scalar if blk % 2 else nc.sync
                eoag_dma = q.dma_start(
                    out=eo_f, in_=ag_out[half][rk * 128:(rk + 1) * 128, :])
                # pin each queue's first eoag after stage-1's LAST PSUM->SBUF
                # copies and the operand loads, then keep block order within
                # the queue.  Without this the scheduler hoists an AG-gated
                # dispatch ahead of the half-1 copies: the engine FIFO
                # head-of-line blocks on the collective, PSUM turnaround
                # stalls, and every core's stage-1 tail (so the collective
                # itself) serializes (measured 68us PE stall).
                key = "prev_s" if blk % 2 else "prev_y"
                prev = eoag_state.get(key)
                if prev is not None:
                    add_dep_helper(eoag_dma.ins, prev.ins, False,
                                   "eoag in block order per queue")
                else:
                    for root in (last_copy["dve"], last_copy["act"],
                                 eoag_state.get("root") or last_we[0]):
                        if root is not None:
                            add_dep_helper(eoag_dma.ins, root.ins, False,
                                           "first eoag after stage-1 tail")
                eoag_state[key] = eoag_dma
                return unflat(eo_f, SUB)

            def s2_block(blk):
                eo_t = load_eoag(blk)
                cw_t = cw_tiles[blk]
                for s2i in range(SUB // 2):
                    for i in range(4):
                        mm_pair(psums[i],
                                cw_t[:, 2 * s2i:2 * s2i + 2, i * 128:(i + 1) * 128],
                                lambda j, s2i=s2i: eo_t[:, 2 * s2i:2 * s2i + 2,
                                                        j * NFREE:(j + 1) * NFREE],
                                start=False, stop=False,
                                perf_mode=mybir.MatmulPerfMode.DoubleRow)
                return cw_t, eo_t

            def s2_tail(blks):
                # bank-major sweep over the last blocks: each (i, j0/j1) bank
                # pair finishes ~4.2 us apart, so its PSUM->SBUF copies (split
                # DVE/ACT) and output DMAs (split sync/scalar) hide behind the
                # remaining matmuls instead of stacking up after the last one
                tiles = []
                for b in blks:
                    eo_t = load_eoag(b)
                    tiles.append((cw_tiles[b], eo_t))
                for i in range(4):
                    for bi, (cw_t, eo_t) in enumerate(tiles):
                        for s2i in range(SUB // 2):
                            mm_pair(psums[i],
                                    cw_t[:, 2 * s2i:2 * s2i + 2, i * 128:(i + 1) * 128],
                                    lambda j, s2i=s2i, eo_t=eo_t: eo_t[:, 2 * s2i:2 * s2i + 2,
                                                                      j * NFREE:(j + 1) * NFREE],
                                    start=False,
                                    stop=(bi == len(tiles) - 1 and s2i == SUB // 2 - 1),
                                    perf_mode=mybir.MatmulPerfMode.DoubleRow)
                    for j in range(NJ):
                        if SPLIT_COPY and j % 2:
                            nc.scalar.copy(out_sb[:, i, j * NFREE:(j + 1) * NFREE],
                                           psums[i][j])
                        else:
                            nc.vector.tensor_copy(
                                out=out_sb[:, i, j * NFREE:(j + 1) * NFREE],
                                in_=psums[i][j])
                        q = nc.scalar if (SPLIT_COPY and j % 2) else nc.sync
                        q.dma_start(
                            out=out[i * 128:(i + 1) * 128, j * NFREE:(j + 1) * NFREE]
                                .rearrange("(n p) d -> p n d", p=128),
                            in_=out_sb[:, i:i + 1, j * NFREE:(j + 1) * NFREE])

            def s3_blocks(b0, b1, start):
                if b0 == 0:
                    # issue the remaining stage-3 operand loads now, BEFORE any
                    # AG-gated eoag dispatch enters the rings (ring descriptors
                    # process in order; anything queued behind a gated eoag
                    # waits for the collective)
                    for blk in range(NBLK):
                        if blk not in ri_tiles:
                            load_ri(blk)
                        if blk not in rw_tiles:
                            eoag_state["root"] = load_rw(blk)
                for blk in range(b0, b1):
                    if blk not in ri_tiles:
                        load_ri(blk)
                    ri_t = ri_tiles[blk]
                    if blk not in rw_tiles:
                        eoag_state["root"] = load_rw(blk)
                    rw_t = rw_tiles[blk]
                    if s3_dr:
                        for s2 in range(SUB // 2):
                            for i in range(4):
                                mm_pair(psums[i],
                                        ri_t[:, 2 * s2:2 * s2 + 2, i * 128:(i + 1) * 128],
                                        lambda j, s2=s2: rw_t[:, 2 * s2:2 * s2 + 2,
                                                              j * NFREE:(j + 1) * NFREE],
                                        start=(start and blk == b0 and s2 == 0),
                                        stop=False,
                                        perf_mode=mybir.MatmulPerfMode.DoubleRow)
                    else:
                        for sub in range(SUB):
                            for i in range(4):
                                mm_pair(psums[i],
                                        ri_t[:, sub, i * 128:(i + 1) * 128],
                                        lambda j, sub=sub: rw_t[:, sub, j * NFREE:(j + 1) * NFREE],
                                        start=(start and blk == b0 and sub == 0),
                                        stop=False)

            s3_blocks(0, S3_SPLIT, start=True)       # covers AG0's tail
            if LOCAL_BF16:
                s2_local_block(0)                    # AG-independent filler
            for blk in range(8):                     # stage-2 fp8 half 0
                s2_block(blk)
            s3_blocks(S3_SPLIT, NBLK, start=False)   # covers AG1's tail
            if LOCAL_BF16 == 2:
                s2_local_block(1)                    # AG-independent filler
            for blk in range(8, 10):                 # stage-2 fp8 half 1
                s2_block(blk)
            s2_tail(list(range(10, ECT // SUB)))     # bank-major finish

    nc.finalize()
    if PAIR_SKIP:
        d, n_del = _pairskip_ldweights_json(json.loads(nc.to_json_bytes()))
        fused = json.dumps(d).encode()
        nc.to_json_bytes = lambda: fused
    return nc


def _get_prog(ldw_opt):
    key = (ldw_opt,)
    if key not in _prog_cache:
        if ldw_opt:
            _patch_ldw_opt()
        _prog_cache[key] = _build(ldw_opt)
    return _prog_cache[key]


def _swz(a, nblk, nsub):
    """[nblk*nsub*128, d] contraction-major -> partition-major [128, nblk*nsub*d]
    with each partition's data contiguous (fat DMA descriptors)."""
    d = a.shape[1]
    return np.ascontiguousarray(
        a.reshape(nblk, nsub, 128, d).transpose(2, 0, 1, 3).reshape(128, nblk * nsub * d))


def _prep_in_maps(inputs, expert_w, residual_w, combine_weights, residual_weight):
    np_dt = BF16
    front = inputs[:E * C].reshape(E, C, D_IN)
    resid = inputs[E * C:]                       # [TOK, D_IN]
    rwt = residual_weight.reshape(TOK, 2)
    w0, w1 = rwt[:, 0], rwt[:, 1]

    s3_np = F8 if FP8_S3 else np_dt
    rw_scale = FP8_S if FP8_S3 else 1.0
    rw_sw = _swz((residual_w * rw_scale).astype(s3_np), 8, 4)            # [128, 8*4*1024]
    resid_s = resid * (w1[:, None] / rw_scale)   # fold w1 and 1/S (fp32)
    in_maps = []
    for r in range(N_CORES):
        sl = slice(r * S_LOC, (r + 1) * S_LOC)
        fT = front[r].T.astype(np_dt)                                    # [D_IN, C]
        # fT layout [128, (ch, blk, sub, c)]: the kernel reads c-halves
        fT_sw = np.ascontiguousarray(
            fT.reshape(16, 2, 128, 2, CH).transpose(2, 3, 0, 1, 4).reshape(128, -1))
        we_sw = _swz(expert_w[r].astype(np_dt), 16, 2)                   # [128, 16*2*1024]
        # centered combine weights: v = (cw - 0.5) * w0; the removed mean is
        # added back exactly on the host (rank-1 term, see _run)
        cw_c = (combine_weights[sl] - 0.5) * w0[sl, None, None]          # [S_LOC, E, C]
        # contraction rows ordered (c-half chunk, expert, c-within-half) to
        # match the chunked AllGather's concatenation
        cwT_f32 = (cw_c.reshape(S_LOC, E, 2, CH).transpose(2, 1, 3, 0)
                   .reshape(E * C, S_LOC))
        cw8_src = cwT_f32 * S2_SCALE
        if LOCAL_BF16:
            # this core's own chunks run through the local bf16 path; zero
            # their fp8 slots (the SPMD program still executes them)
            loc_blocks = [r, 8 + r][:LOCAL_BF16]
            cw8_src = cw8_src.copy()
            for b in loc_blocks:
                cw8_src[b * 512:(b + 1) * 512] = 0.0
            cw16 = np.concatenate(
                [cwT_f32[b * 512:(b + 1) * 512]
                 for b in sorted(loc_blocks)], axis=0)
            cwT16_sw = _swz(cw16.astype(np_dt), LOCAL_BF16, 4)
        cwT8_sw = _swz(cw8_src.astype(F8), 16, 4)                        # [128, 16*4*512]
        riT_sw = _swz(resid_s[sl].T.astype(s3_np), 8, 4)                 # [128, 8*4*512]
        m = {"fT": fT_sw, "we": we_sw, "cwT8": cwT8_sw, "riT": riT_sw, "rw": rw_sw}
        if LOCAL_BF16:
            m["cwT16"] = cwT16_sw
        in_maps.append(m)
    return in_maps


def _run(inputs, expert_w, expert_b, residual_w, residual_b,
         combine_weights, residual_weight, ldw_opt=None, trace=False, mode=None):
    import jax
    try:
        if jax.config.jax_compilation_cache_dir is None:
            jax.config.update("jax_compilation_cache_dir", "/tmp/jax_cache_trn_moe")
            jax.config.update("jax_persistent_cache_min_compile_time_secs", 0.5)
    except Exception:
        pass
    from concourse.bass_utils import run_bass_kernel_spmd

    ldw_opt = LDW_OPT if ldw_opt is None else ldw_opt
    inputs = np.asarray(inputs, dtype=np.float32)
    expert_w = np.asarray(expert_w, dtype=np.float32)
    expert_b = np.asarray(expert_b, dtype=np.float32)
    residual_w = np.asarray(residual_w, dtype=np.float32)
    residual_b = np.asarray(residual_b, dtype=np.float32)
    combine_weights = np.asarray(combine_weights, dtype=np.float32)
    residual_weight = np.asarray(residual_weight, dtype=np.float32)

    nc = _get_prog(ldw_opt)
    in_maps = _prep_in_maps(inputs, expert_w, residual_w, combine_weights,
                            residual_weight)
    res = run_bass_kernel_spmd(nc, in_maps, list(range(N_CORES)), trace=trace)
    out = np.concatenate([res.results[r]["out"] for r in range(N_CORES)], axis=0)

    rwt = residual_weight.reshape(TOK, 2)
    # exact centering term: 0.5 * w0[s] * colsum(front @ W)[m], float64
    front = inputs[:E * C].reshape(E, C, D_IN)
    G = np.zeros(D_OUT, np.float64)
    for e in range(E):
        G += front[e].sum(axis=0).astype(np.float64) @ expert_w[e].astype(np.float64)
    out = out + (0.5 * rwt[:, 0:1]) * G[None, :].astype(np.float32)
    # exact bias contributions (zero in practice, but keep the math honest)
    if residual_b.any():
        out = out + rwt[:, 1:2] * residual_b[None, :]
    if expert_b.any():
        cs = combine_weights.sum(axis=2)                    # [TOK, E]
        out = out + rwt[:, 0:1] * (cs @ expert_b)
    return out.reshape(B, S, D_OUT).astype(np.float32), res


def kernel(**kw):
    out, _ = _run(**kw)
    return out
